# revision 4
# baseline (speedup 1.0000x reference)
"""Trainium2 Bass kernel for CausalSelfAttention (B=1, T=2048, C=4096,
32 heads / 8 query groups / head_size 128, full-dim RoPE, GQA).

Sharding: tensor-parallel over the 8 query groups. Core g owns w_attn rows
[g*768:(g+1)*768] (4 q heads + 1 k + 1 v) and w_proj columns
[g*512:(g+1)*512]; x is replicated. Each core returns a bf16 partial
projection output [2048, 4096]; the host sums the 8 partials.

Matmul precision strategy:
- QKV projection and output projection run as fp8e4m3 DoubleRow matmuls
  with a 3-term split (w = wh+wl, x = xh+xl; wh@xh + wh@xl + wl@xh), with
  weights pre-scaled by 64 on the host to avoid fp8 subnormals.
- Attention (QK^T, P@V) runs in bf16; softmax denominator via pair-summed
  P tiles (bf16 adds on DVE) + ones-matmuls into PSUM.
"""

import os
import sys

for _p in ("/opt/trn_rl_repo", "/root/.axon_site/_ro/trn_rl_repo"):
    if os.path.isdir(_p) and _p not in sys.path:
        sys.path.insert(0, _p)

import numpy as np
import ml_dtypes

import concourse.bass as bass
import concourse.mybir as mybir
import concourse.tile as tile
from concourse import bacc, bass_utils

N_CORES = 8
T = 2048
C = 4096
HS = 128
G = 8                      # query groups == cores
QPK = 4                    # q heads per group
NCOMP = QPK + 2            # q0..q3, k, v
RG = NCOMP * HS            # 768 w_attn rows per group
OG = QPK * HS              # 512 proj-input cols per group
NT = T // 512              # 4 blocks of 512 along t
NC = C // 128              # 32 contraction chunks
NM = NC // 2               # 16 chunk-pairs
SCALE = 1.0 / np.sqrt(float(HS))
WSCALE = 64.0              # host pre-scale for w_attn / w_proj fp8

F32 = mybir.dt.float32
F32R = mybir.dt.float32r
BF16 = mybir.dt.bfloat16
FP8 = mybir.dt.float8e4
DR = mybir.MatmulPerfMode.DoubleRow


def _build_program():
    nc = bacc.Bacc(trn_type="TRN2", target_bir_lowering=False, debug=False,
                   num_devices=N_CORES)

    d_xh = nc.dram_tensor("xh", [C, T], FP8, kind="ExternalInput").ap()
    d_xl = nc.dram_tensor("xl", [C, T], FP8, kind="ExternalInput").ap()
    d_wh = nc.dram_tensor("wh", [C, RG], FP8, kind="ExternalInput").ap()
    d_wl = nc.dram_tensor("wl", [C, RG], FP8, kind="ExternalInput").ap()
    d_wph = nc.dram_tensor("wph", [OG, C], FP8, kind="ExternalInput").ap()
    d_wpl = nc.dram_tensor("wpl", [OG, C], FP8, kind="ExternalInput").ap()
    d_cos = nc.dram_tensor("cost", [HS, T], F32R, kind="ExternalInput").ap()
    d_sin = nc.dram_tensor("sint", [HS, T], F32R, kind="ExternalInput").ap()
    d_mask = nc.dram_tensor("mask", [128, 128], BF16,
                            kind="ExternalInput").ap()
    d_perm = nc.dram_tensor("perm", [128, 128], F32R, kind="ExternalInput").ap()
    d_idn = nc.dram_tensor("idn", [128, 128], BF16, kind="ExternalInput").ap()
    d_onesb = nc.dram_tensor("onesb", [128, 128], BF16,
                             kind="ExternalInput").ap()
    d_onesr = nc.dram_tensor("onesr", [128, 128], F32R,
                             kind="ExternalInput").ap()
    d_out = nc.dram_tensor("out", [T, C], BF16, kind="ExternalOutput").ap()

    with tile.TileContext(nc) as tc:
        with tc.tile_pool(name="glob", bufs=1) as glob:
            # roped q0..q3 / k in bf16, one tile per (comp, t-block)
            QQ = [[glob.tile([128, 512], BF16, name=f"qq{j}_{tb}",
                             tag=f"qq{j}_{tb}")
                   for tb in range(NT)] for j in range(5)]
            # V in [t, hs] layout (bf16), col u = t-chunk
            V = [glob.tile([128, 512], BF16, name=f"v{tb}", tag=f"v{tb}")
                 for tb in range(NT)]
            # proj inputs: fp8 hi/lo, head-paired for DoubleRow
            YA = [glob.tile([128, 2, T], FP8, name=f"ya{i}", tag=f"ya{i}")
                  for i in range(2)]
            YB = [glob.tile([128, 2, T], FP8, name=f"yb{i}", tag=f"yb{i}")
                  for i in range(2)]
            COS = glob.tile([128, T], F32R)
            SIN = glob.tile([128, T], F32R)
            MASK = glob.tile([128, 128], BF16)
            PERM = glob.tile([128, 128], F32R)
            IDN = glob.tile([128, 128], BF16)
            ONESB = glob.tile([128, 128], BF16)
            ONESR = glob.tile([128, 128], F32R)

            nc.sync.dma_start(COS[:], d_cos[:])
            nc.sync.dma_start(SIN[:], d_sin[:])
            nc.sync.dma_start(MASK[:], d_mask[:])
            nc.sync.dma_start(PERM[:], d_perm[:])
            nc.sync.dma_start(IDN[:], d_idn[:])
            nc.sync.dma_start(ONESB[:], d_onesb[:])
            nc.sync.dma_start(ONESR[:], d_onesr[:])

            # ---------------- Phase A: qkv projection + rope -------------
            with tc.tile_pool(name="wa", bufs=1) as wap, \
                 tc.tile_pool(name="xp", bufs=2) as xp, \
                 tc.tile_pool(name="tmpa", bufs=2) as tmpa, \
                 tc.tile_pool(name="psA", bufs=1, space="PSUM") as psA, \
                 tc.tile_pool(name="psR", bufs=2, space="PSUM") as psR:
                WH = [wap.tile([128, 2, RG], FP8, name=f"wh{m}", tag=f"wh{m}")
                      for m in range(NM)]
                WL = [wap.tile([128, 2, RG], FP8, name=f"wl{m}", tag=f"wl{m}")
                      for m in range(NM)]

                for tb in range(NT):
                    ts = slice(tb * 512, (tb + 1) * 512)
                    qkv_ps = [psA.tile([128, 512], F32, tag=f"qkv{j}",
                                       name=f"qkv{j}")
                              for j in range(NCOMP)]
                    XH = [xp.tile([128, 2, 512], FP8, name=f"xht{m}", tag=f"xh{m}")
                          for m in range(NM)]
                    XL = [xp.tile([128, 2, 512], FP8, name=f"xlt{m}", tag=f"xl{m}")
                          for m in range(NM)]
                    for m in range(NM):
                        for i in range(2):
                            n = 2 * m + i
                            if tb == 0:
                                nc.sync.dma_start(
                                    WH[m][:, i, :],
                                    d_wh[n * 128:(n + 1) * 128, :])
                                nc.sync.dma_start(
                                    WL[m][:, i, :],
                                    d_wl[n * 128:(n + 1) * 128, :])
                            nc.sync.dma_start(
                                XH[m][:, i, :],
                                d_xh[n * 128:(n + 1) * 128, ts])
                            nc.sync.dma_start(
                                XL[m][:, i, :],
                                d_xl[n * 128:(n + 1) * 128, ts])
                    for j in range(NCOMP):
                        js = slice(j * HS, (j + 1) * HS)
                        for m in range(NM):
                            for t_i, (wt, xt) in enumerate(
                                    ((WH, XH), (WH, XL), (WL, XH))):
                                nc.tensor.matmul(
                                    qkv_ps[j][:],
                                    wt[m][:, :, js],
                                    xt[m][:, :, :],
                                    start=(m == 0 and t_i == 0),
                                    stop=(m == NM - 1 and t_i == 2),
                                    perf_mode=DR)

                    for j in range(5):  # q0..q3, k get rope
                        raw = tmpa.tile([128, 512], F32R, tag="raw")
                        nc.scalar.copy(raw[:], qkv_ps[j][:])
                        rot = psR.tile([128, 512], F32, tag="rot")
                        nc.tensor.matmul(rot[:], PERM[:], raw[:],
                                         start=True, stop=True)
                        t1 = tmpa.tile([128, 512], F32R, tag="t1")
                        nc.vector.tensor_tensor(t1[:], raw[:], COS[:, ts],
                                                mybir.AluOpType.mult)
                        t2 = tmpa.tile([128, 512], F32R, tag="t2")
                        nc.vector.tensor_tensor(t2[:], rot[:], SIN[:, ts],
                                                mybir.AluOpType.mult)
                        nc.vector.tensor_tensor(QQ[j][tb][:], t1[:], t2[:],
                                                mybir.AluOpType.add)

                    # v: scale back by 1/64, transpose [hs, t] -> [t, hs]
                    vraw = tmpa.tile([128, 512], BF16, tag="vraw")
                    nc.scalar.mul(vraw[:], qkv_ps[5][:], 1.0 / WSCALE)
                    for u in range(4):
                        vt = psR.tile([128, 128], BF16, tag="rot")
                        nc.tensor.transpose(vt[:],
                                            vraw[:, u * 128:(u + 1) * 128],
                                            IDN[:])
                        nc.vector.tensor_copy(
                            V[tb][:, u * 128:(u + 1) * 128], vt[:])

            # ---------------- Phase B: causal attention ------------------
            with tc.tile_pool(name="wp", bufs=1) as wpp, \
                 tc.tile_pool(name="pwp", bufs=4) as pwp, \
                 tc.tile_pool(name="prp", bufs=2) as prp, \
                 tc.tile_pool(name="bcp", bufs=2) as bcp, \
                 tc.tile_pool(name="yfp", bufs=2) as yfp, \
                 tc.tile_pool(name="rcp", bufs=2) as rcp:
                WPH = [wpp.tile([128, 2, C], FP8, name=f"wph{i}",
                                tag=f"wph{i}") for i in range(2)]
                WPL = [wpp.tile([128, 2, C], FP8, name=f"wpl{i}",
                                tag=f"wpl{i}") for i in range(2)]
                for h in range(QPK):
                    nc.sync.dma_start(WPH[h // 2][:, h % 2, :],
                                      d_wph[h * 128:(h + 1) * 128, :])
                    nc.sync.dma_start(WPL[h // 2][:, h % 2, :],
                                      d_wpl[h * 128:(h + 1) * 128, :])

                with tc.tile_pool(name="psS", bufs=2, space="PSUM") as psS, \
                     tc.tile_pool(name="psY", bufs=2, space="PSUM") as psY, \
                     tc.tile_pool(name="psD", bufs=1, space="PSUM") as psD, \
                     tc.tile_pool(name="psB", bufs=1, space="PSUM") as psB:
                    for h in range(QPK):
                        for b in range(NT):
                            nkt = 4 * (b + 1)
                            y_ps = psY.tile([128, 512], F32, tag="y")
                            d_ps = psD.tile([1, 512], F32, tag="d")
                            first_d = [True]
                            for u in range(nkt // 2):
                                s_w = psS.tile([128, 1024], F32, tag="s")
                                p_w = pwp.tile([128, 1024], BF16, tag="p")
                                offs = []
                                for half in range(2):
                                    kt = 2 * u + half
                                    r = kt - 4 * b
                                    off = 0 if r < 0 else r * 128
                                    offs.append(off)
                                    co = half * 512
                                    nc.tensor.matmul(
                                        s_w[:, co + off:co + 512],
                                        QQ[4][kt // 4][:, (kt % 4) * 128:
                                                       (kt % 4 + 1) * 128],
                                        QQ[h][b][:, off:],
                                        start=True, stop=True)
                                full_pair = offs[0] == 0 and offs[1] == 0
                                if full_pair:
                                    nc.scalar.activation(
                                        p_w[:], s_w[:],
                                        mybir.ActivationFunctionType.Exp,
                                        scale=SCALE)
                                else:
                                    for half in range(2):
                                        co = half * 512 + offs[half]
                                        nc.scalar.activation(
                                            p_w[:, co:half * 512 + 512],
                                            s_w[:, co:half * 512 + 512],
                                            mybir.ActivationFunctionType.Exp,
                                            scale=SCALE)
                                for half in range(2):
                                    kt = 2 * u + half
                                    r = kt - 4 * b
                                    if r >= 0:  # diagonal: mask 128-col strip
                                        co = half * 512 + r * 128
                                        nc.vector.tensor_tensor(
                                            p_w[:, co:co + 128],
                                            p_w[:, co:co + 128],
                                            MASK[:],
                                            mybir.AluOpType.mult)
                                for half in range(2):
                                    kt = 2 * u + half
                                    off = offs[half]
                                    co = half * 512
                                    nc.tensor.matmul(
                                        y_ps[:, off:],
                                        V[kt // 4][:, (kt % 4) * 128:
                                                   (kt % 4 + 1) * 128],
                                        p_w[:, co + off:co + 512],
                                        start=(kt == 0), stop=(kt == nkt - 1))
                                # denominator
                                if full_pair:
                                    pr = prp.tile([128, 512], BF16, tag="pr")
                                    nc.vector.tensor_tensor(
                                        pr[:], p_w[:, :512], p_w[:, 512:],
                                        mybir.AluOpType.add)
                                    nc.tensor.matmul(
                                        d_ps[:], ONESB[:, :1], pr[:],
                                        start=first_d[0], stop=(u == nkt // 2 - 1),
                                        skip_group_check=True)
                                    first_d[0] = False
                                else:
                                    for half in range(2):
                                        off = offs[half]
                                        co = half * 512
                                        nc.tensor.matmul(
                                            d_ps[:, off:], ONESB[:, :1],
                                            p_w[:, co + off:co + 512],
                                            start=first_d[0],
                                            stop=(u == nkt // 2 - 1 and
                                                  half == 1))
                                        first_d[0] = False
                            recip = rcp.tile([1, 512], F32R, tag="r")
                            with nc.allow_low_precision(
                                    reason="f32r is f32-width"):
                                nc.vector.reciprocal(recip[:], d_ps[:])
                            bc_ps = psB.tile([128, 512], F32, tag="bc")
                            nc.tensor.matmul(bc_ps[:], ONESR[:1, :], recip[:],
                                             start=True, stop=True)
                            bc_sb = bcp.tile([128, 512], BF16, tag="bc")
                            nc.vector.tensor_copy(bc_sb[:], bc_ps[:])
                            yf = yfp.tile([128, 512], F32R, tag="yf")
                            nc.vector.tensor_tensor(yf[:], y_ps[:], bc_sb[:],
                                                    mybir.AluOpType.mult)
                            bs = slice(b * 512, (b + 1) * 512)
                            nc.vector.tensor_copy(
                                YA[h // 2][:, h % 2, bs], yf[:])
                            nc.vector.tensor_tensor(
                                YB[h // 2][:, h % 2, bs], yf[:],
                                YA[h // 2][:, h % 2, bs],
                                mybir.AluOpType.subtract)

                # ---------------- Phase C: output projection -------------
                with tc.tile_pool(name="outp", bufs=4) as outp, \
                     tc.tile_pool(name="psO", bufs=4, space="PSUM") as psO:
                    for tt in range(T // 128):
                        tsl = slice(tt * 128, (tt + 1) * 128)
                        for cb in range(C // 512):
                            cs = slice(cb * 512, (cb + 1) * 512)
                            o_ps = psO.tile([128, 512], F32, tag="o")
                            terms = [(YA[0], WPH[0]), (YA[1], WPH[1]),
                                     (YB[0], WPH[0]), (YB[1], WPH[1]),
                                     (YA[0], WPL[0]), (YA[1], WPL[1])]
                            for t_i, (yt, wt) in enumerate(terms):
                                nc.tensor.matmul(
                                    o_ps[:], yt[:, :, tsl], wt[:, :, cs],
                                    start=(t_i == 0), stop=(t_i == 5),
                                    perf_mode=DR)
                            o_sb = outp.tile([128, 512], BF16, tag="o")
                            nc.scalar.mul(o_sb[:], o_ps[:], 1.0 / WSCALE)
                            nc.sync.dma_start(d_out[tsl, cs], o_sb[:])
    nc.compile()
    return nc


def _host_inputs(x, cos, sin, w_attn, w_proj):
    """Build per-core input maps (host-side shard/transpose/fp8 split)."""
    f = np.float32
    f8 = ml_dtypes.float8_e4m3fn

    def split8(a):
        hi = a.astype(f8)
        lo = (a - hi.astype(f)).astype(f8)
        return np.ascontiguousarray(hi), np.ascontiguousarray(lo)

    xt = x.reshape(T, C).T.astype(f)                         # [C, T]
    xh, xl = split8(xt)
    cost = np.ascontiguousarray(cos.T.astype(f) / WSCALE)    # [HS, T]
    sgn = np.ones((HS, 1), f)
    sgn[:HS // 2] = -1.0
    sint = np.ascontiguousarray((sin.T * sgn).astype(f) / WSCALE)
    # rot(x)=P@x in [d,t] layout; matmul computes lhsT.T @ rhs -> lhsT = P.T
    P = np.zeros((HS, HS), f)
    for i in range(HS // 2):
        P[i, i + HS // 2] = 1.0
        P[i + HS // 2, i] = 1.0
    perm = np.ascontiguousarray(P.T)
    idn = np.eye(128, dtype=f).astype(ml_dtypes.bfloat16)
    onesb = np.ones((128, 128), ml_dtypes.bfloat16)
    onesr = np.ones((128, 128), f)
    # causal mask strip: within a diagonal 128x128 tile, q offset i vs key
    # partition p: keep iff i >= p (same strip for every diagonal offset r)
    pidx = np.arange(128)
    cols = np.arange(128)
    mask = (cols[None, :] >= pidx[:, None]).astype(f).astype(
        ml_dtypes.bfloat16)

    maps = []
    for g in range(N_CORES):
        wat = (w_attn[g * RG:(g + 1) * RG, :].T.astype(f) * WSCALE)  # [C, RG]
        wh, wl = split8(wat)
        wpt = (w_proj[:, g * OG:(g + 1) * OG].T.astype(f) * WSCALE)  # [OG, C]
        wph, wpl = split8(wpt)
        maps.append({
            "xh": xh, "xl": xl, "wh": wh, "wl": wl, "wph": wph, "wpl": wpl,
            "cost": cost, "sint": sint, "mask": mask, "perm": perm,
            "idn": idn, "onesb": onesb, "onesr": onesr,
        })
    return maps


_PROGRAM = None


def kernel(x, cos, sin, w_attn, w_proj):
    global _PROGRAM
    if _PROGRAM is None:
        _PROGRAM = _build_program()
    maps = _host_inputs(np.asarray(x), np.asarray(cos), np.asarray(sin),
                        np.asarray(w_attn), np.asarray(w_proj))
    res = bass_utils.run_bass_kernel_spmd(_PROGRAM, maps, list(range(N_CORES)))
    out = np.zeros((T, C), np.float32)
    for g in range(N_CORES):
        out += np.asarray(res.results[g]["out"]).astype(np.float32)
    return out.reshape(1, T, C)


# revision 5
# speedup vs baseline: 1.2873x; 1.2873x over previous
"""Trainium2 Bass kernel for CausalSelfAttention (B=1, T=2048, C=4096,
32 heads / 8 query groups / head_size 128, full-dim RoPE, GQA).

Sharding: tensor-parallel over the 8 query groups. Core g owns w_attn rows
[g*768:(g+1)*768] (4 q heads + 1 k + 1 v) and w_proj columns
[g*512:(g+1)*512]; x is replicated. Each core returns a bf16 partial
projection output [2048, 4096]; the host sums the 8 partials.

Matmul precision strategy:
- QKV projection and output projection run as fp8e4m3 DoubleRow matmuls
  with a 3-term split (w = wh+wl, x = xh+xl; wh@xh + wh@xl + wl@xh), with
  weights pre-scaled by 64 on the host to avoid fp8 subnormals.
- Attention (QK^T, P@V) runs in bf16; softmax denominator via pair/quad
  summed P tiles (bf16 adds on DVE) + ones-matmuls into PSUM.
- All DMA transfers are batched (contiguous-per-partition host layouts):
  each dma_start costs ~625ns of serialized descriptor generation
  regardless of size, so few big transfers beat many small ones.
"""

import os
import sys

for _p in ("/opt/trn_rl_repo", "/root/.axon_site/_ro/trn_rl_repo"):
    if os.path.isdir(_p) and _p not in sys.path:
        sys.path.insert(0, _p)

import numpy as np
import ml_dtypes

import concourse.bass as bass
import concourse.mybir as mybir
import concourse.tile as tile
from concourse import bacc, bass_utils

N_CORES = 8
T = 2048
C = 4096
HS = 128
G = 8                      # query groups == cores
QPK = 4                    # q heads per group
NCOMP = QPK + 2            # q0..q3, k, v
RG = NCOMP * HS            # 768 w_attn rows per group
OG = QPK * HS              # 512 proj-input cols per group
NT = T // 512              # 4 blocks of 512 along t
NC = C // 128              # 32 contraction chunks
NM = NC // 2               # 16 chunk-pairs
NQ = 4                     # quarters (4 chunk-pairs each) per contraction
SCALE = 1.0 / np.sqrt(float(HS))
WSCALE = 64.0              # host pre-scale for w_attn / w_proj fp8

F32 = mybir.dt.float32
F32R = mybir.dt.float32r
BF16 = mybir.dt.bfloat16
FP8 = mybir.dt.float8e4
DR = mybir.MatmulPerfMode.DoubleRow


def _build_program():
    nc = bacc.Bacc(trn_type="TRN2", target_bir_lowering=False, debug=False,
                   num_devices=N_CORES)

    # x splits laid out [tb, p, n, c]; w splits laid out [p, n, col]
    d_xh = nc.dram_tensor("xh", [NT, 128, NC, 512], FP8,
                          kind="ExternalInput").ap()
    d_xl = nc.dram_tensor("xl", [NT, 128, NC, 512], FP8,
                          kind="ExternalInput").ap()
    d_wh = nc.dram_tensor("wh", [128, NC, RG], FP8, kind="ExternalInput").ap()
    d_wl = nc.dram_tensor("wl", [128, NC, RG], FP8, kind="ExternalInput").ap()
    d_wph = nc.dram_tensor("wph", [OG, C], FP8, kind="ExternalInput").ap()
    d_wpl = nc.dram_tensor("wpl", [OG, C], FP8, kind="ExternalInput").ap()
    d_cos = nc.dram_tensor("cost", [HS, T], F32R, kind="ExternalInput").ap()
    d_sin = nc.dram_tensor("sint", [HS, T], F32R, kind="ExternalInput").ap()
    d_mask = nc.dram_tensor("mask", [128, 128], BF16,
                            kind="ExternalInput").ap()
    d_perm = nc.dram_tensor("perm", [128, 128], F32R,
                            kind="ExternalInput").ap()
    d_idn = nc.dram_tensor("idn", [128, 128], BF16, kind="ExternalInput").ap()
    d_onesb = nc.dram_tensor("onesb", [128, 128], BF16,
                             kind="ExternalInput").ap()
    d_onesr = nc.dram_tensor("onesr", [128, 128], F32R,
                             kind="ExternalInput").ap()
    d_out = nc.dram_tensor("out", [T, C], BF16, kind="ExternalOutput").ap()

    with tile.TileContext(nc) as tc:
        with tc.tile_pool(name="glob", bufs=1) as glob:
            # roped q0..q3 / k in bf16, one tile per (comp, t-block)
            QQ = [[glob.tile([128, 512], BF16, name=f"qq{j}_{tb}",
                             tag=f"qq{j}_{tb}")
                   for tb in range(NT)] for j in range(5)]
            # V in [t, hs] layout (bf16), col u = t-chunk
            V = [glob.tile([128, 512], BF16, name=f"v{tb}", tag=f"v{tb}")
                 for tb in range(NT)]
            # proj inputs: fp8 hi/lo, head-paired, one tile per (pair, b)
            YA = [[glob.tile([128, 2, 512], FP8, name=f"ya{i}_{b}",
                             tag=f"ya{i}_{b}") for b in range(NT)]
                  for i in range(2)]
            YB = [[glob.tile([128, 2, 512], FP8, name=f"yb{i}_{b}",
                             tag=f"yb{i}_{b}") for b in range(NT)]
                  for i in range(2)]
            COS = glob.tile([128, T], F32R)
            SIN = glob.tile([128, T], F32R)
            MASK = glob.tile([128, 128], BF16)
            PERM = glob.tile([128, 128], F32R)
            IDN = glob.tile([128, 128], BF16)
            ONESB = glob.tile([128, 128], BF16)
            ONESR = glob.tile([128, 128], F32R)

            nc.sync.dma_start(COS[:], d_cos[:])
            nc.sync.dma_start(SIN[:], d_sin[:])
            nc.sync.dma_start(MASK[:], d_mask[:])
            nc.sync.dma_start(PERM[:], d_perm[:])
            nc.sync.dma_start(IDN[:], d_idn[:])
            nc.sync.dma_start(ONESB[:], d_onesb[:])
            nc.sync.dma_start(ONESR[:], d_onesr[:])

            # ---------------- Phase A: qkv projection + rope -------------
            with tc.tile_pool(name="wa", bufs=1) as wap, \
                 tc.tile_pool(name="xp", bufs=2) as xp, \
                 tc.tile_pool(name="tmpa", bufs=2) as tmpa, \
                 tc.tile_pool(name="psA", bufs=1, space="PSUM") as psA, \
                 tc.tile_pool(name="psR", bufs=2, space="PSUM") as psR:
                WH = [wap.tile([128, 8, RG], FP8, name=f"whq{q}",
                               tag=f"wh{q}") for q in range(NQ)]
                WL = [wap.tile([128, 8, RG], FP8, name=f"wlq{q}",
                               tag=f"wl{q}") for q in range(NQ)]

                for tb in range(NT):
                    ts = slice(tb * 512, (tb + 1) * 512)
                    qkv_ps = [psA.tile([128, 512], F32, tag=f"qkv{j}",
                                       name=f"qkv{j}")
                              for j in range(NCOMP)]
                    XH = [xp.tile([128, 8, 512], FP8, name=f"xhq{q}",
                                  tag=f"xh{q}") for q in range(NQ)]
                    XL = [xp.tile([128, 8, 512], FP8, name=f"xlq{q}",
                                  tag=f"xl{q}") for q in range(NQ)]
                    for q in range(NQ):
                        cs = slice(q * 8, (q + 1) * 8)
                        if tb == 0:
                            nc.sync.dma_start(WH[q][:], d_wh[:, cs, :])
                            nc.sync.dma_start(WL[q][:], d_wl[:, cs, :])
                        nc.sync.dma_start(XH[q][:], d_xh[tb, :, cs, :])
                        nc.sync.dma_start(XL[q][:], d_xl[tb, :, cs, :])
                    # m-outer so PE consumption tracks the quarter DMAs
                    for m in range(NM):
                        q, u = m // 4, m % 4
                        ps = slice(2 * u, 2 * u + 2)
                        for j in range(NCOMP):
                            js = slice(j * HS, (j + 1) * HS)
                            for t_i, (wt, xt) in enumerate(
                                    ((WH, XH), (WH, XL), (WL, XH))):
                                nc.tensor.matmul(
                                    qkv_ps[j][:],
                                    wt[q][:, ps, js],
                                    xt[q][:, ps, :],
                                    start=(m == 0 and t_i == 0),
                                    stop=(m == NM - 1 and t_i == 2),
                                    perf_mode=DR)

                    for j in range(5):  # q0..q3, k get rope
                        raw = tmpa.tile([128, 512], F32R, tag="raw")
                        nc.scalar.copy(raw[:], qkv_ps[j][:])
                        rot = psR.tile([128, 512], F32, tag="rot")
                        nc.tensor.matmul(rot[:], PERM[:], raw[:],
                                         start=True, stop=True)
                        t1 = tmpa.tile([128, 512], F32R, tag="t1")
                        nc.vector.tensor_tensor(t1[:], raw[:], COS[:, ts],
                                                mybir.AluOpType.mult)
                        t2 = tmpa.tile([128, 512], F32R, tag="t2")
                        nc.vector.tensor_tensor(t2[:], rot[:], SIN[:, ts],
                                                mybir.AluOpType.mult)
                        nc.vector.tensor_tensor(QQ[j][tb][:], t1[:], t2[:],
                                                mybir.AluOpType.add)

                    # v: scale back by 1/64, transpose [hs, t] -> [t, hs]
                    vraw = tmpa.tile([128, 512], BF16, tag="vraw")
                    nc.scalar.mul(vraw[:], qkv_ps[5][:], 1.0 / WSCALE)
                    for u in range(4):
                        vt = psR.tile([128, 128], BF16, tag="rot")
                        nc.tensor.transpose(vt[:],
                                            vraw[:, u * 128:(u + 1) * 128],
                                            IDN[:])
                        nc.vector.tensor_copy(
                            V[tb][:, u * 128:(u + 1) * 128], vt[:])

            # ---------------- Phase B: causal attention ------------------
            with tc.tile_pool(name="wp", bufs=1) as wpp, \
                 tc.tile_pool(name="pwp", bufs=4) as pwp, \
                 tc.tile_pool(name="prp", bufs=3) as prp, \
                 tc.tile_pool(name="bcp", bufs=2) as bcp, \
                 tc.tile_pool(name="yfp", bufs=2) as yfp, \
                 tc.tile_pool(name="orw", bufs=2) as orw, \
                 tc.tile_pool(name="rcp", bufs=2) as rcp:
                WPH = [wpp.tile([128, 2, C], FP8, name=f"wph{i}",
                                tag=f"wph{i}") for i in range(2)]
                WPL = [wpp.tile([128, 2, C], FP8, name=f"wpl{i}",
                                tag=f"wpl{i}") for i in range(2)]
                for h in range(QPK):
                    nc.sync.dma_start(WPH[h // 2][:, h % 2, :],
                                      d_wph[h * 128:(h + 1) * 128, :])
                    nc.sync.dma_start(WPL[h // 2][:, h % 2, :],
                                      d_wpl[h * 128:(h + 1) * 128, :])

                with tc.tile_pool(name="psS", bufs=2, space="PSUM") as psS, \
                     tc.tile_pool(name="psY", bufs=2, space="PSUM") as psY, \
                     tc.tile_pool(name="psD", bufs=1, space="PSUM") as psD, \
                     tc.tile_pool(name="psB", bufs=1, space="PSUM") as psB:
                    for h in range(QPK):
                        for b in range(NT):
                            nkt = 4 * (b + 1)
                            y_ps = psY.tile([128, 512], F32, tag="y")
                            d_ps = psD.tile([1, 512], F32, tag="d")
                            first_d = [True]
                            quad = [None]

                            def d_mm(rhs_ap, off, last):
                                nc.tensor.matmul(
                                    d_ps[:, off:], ONESB[:, :1], rhs_ap,
                                    start=first_d[0], stop=last)
                                first_d[0] = False

                            for u in range(nkt // 2):
                                s_w = psS.tile([128, 1024], F32, tag="s")
                                p_w = pwp.tile([128, 1024], BF16, tag="p")
                                offs = []
                                for half in range(2):
                                    kt = 2 * u + half
                                    r = kt - 4 * b
                                    off = 0 if r < 0 else r * 128
                                    offs.append(off)
                                    co = half * 512
                                    nc.tensor.matmul(
                                        s_w[:, co + off:co + 512],
                                        QQ[4][kt // 4][:, (kt % 4) * 128:
                                                       (kt % 4 + 1) * 128],
                                        QQ[h][b][:, off:],
                                        start=True, stop=True)
                                full_pair = offs[0] == 0 and offs[1] == 0
                                if full_pair:
                                    nc.scalar.activation(
                                        p_w[:], s_w[:],
                                        mybir.ActivationFunctionType.Exp,
                                        scale=SCALE)
                                else:
                                    for half in range(2):
                                        co = half * 512 + offs[half]
                                        nc.scalar.activation(
                                            p_w[:, co:half * 512 + 512],
                                            s_w[:, co:half * 512 + 512],
                                            mybir.ActivationFunctionType.Exp,
                                            scale=SCALE)
                                for half in range(2):
                                    kt = 2 * u + half
                                    r = kt - 4 * b
                                    if r >= 0:  # diagonal: mask 128-col strip
                                        co = half * 512 + r * 128
                                        nc.vector.tensor_tensor(
                                            p_w[:, co:co + 128],
                                            p_w[:, co:co + 128],
                                            MASK[:],
                                            mybir.AluOpType.mult)
                                for half in range(2):
                                    kt = 2 * u + half
                                    off = offs[half]
                                    co = half * 512
                                    nc.tensor.matmul(
                                        y_ps[:, off:],
                                        V[kt // 4][:, (kt % 4) * 128:
                                                   (kt % 4 + 1) * 128],
                                        p_w[:, co + off:co + 512],
                                        start=(kt == 0), stop=(kt == nkt - 1))
                                # denominator: quad-sum full tiles on DVE,
                                # one ones-matmul per quad / diagonal half
                                if full_pair:
                                    pr = prp.tile([128, 512], BF16, tag="pr")
                                    nc.vector.tensor_tensor(
                                        pr[:], p_w[:, :512], p_w[:, 512:],
                                        mybir.AluOpType.add)
                                    if u % 2 == 0:
                                        quad[0] = pr
                                    else:
                                        q2 = prp.tile([128, 512], BF16,
                                                      tag="q2")
                                        nc.vector.tensor_tensor(
                                            q2[:], quad[0][:], pr[:],
                                            mybir.AluOpType.add)
                                        quad[0] = None
                                        d_mm(q2[:], 0, False)
                                else:
                                    if quad[0] is not None:
                                        d_mm(quad[0][:], 0, False)
                                        quad[0] = None
                                    for half in range(2):
                                        off = offs[half]
                                        co = half * 512
                                        d_mm(p_w[:, co + off:co + 512], off,
                                             (u == nkt // 2 - 1 and
                                              half == 1))
                            recip = rcp.tile([1, 512], F32R, tag="r")
                            with nc.allow_low_precision(
                                    reason="f32r is f32-width"):
                                nc.vector.reciprocal(recip[:], d_ps[:])
                            bc_ps = psB.tile([128, 512], F32, tag="bc")
                            nc.tensor.matmul(bc_ps[:], ONESR[:1, :], recip[:],
                                             start=True, stop=True)
                            bc_sb = bcp.tile([128, 512], BF16, tag="bc")
                            nc.vector.tensor_copy(bc_sb[:], bc_ps[:])
                            yf = yfp.tile([128, 512], F32R, tag="yf")
                            nc.vector.tensor_tensor(yf[:], y_ps[:], bc_sb[:],
                                                    mybir.AluOpType.mult)
                            ya = YA[h // 2][b]
                            yb = YB[h // 2][b]
                            nc.vector.tensor_copy(ya[:, h % 2, :], yf[:])
                            nc.vector.tensor_tensor(
                                yb[:, h % 2, :], yf[:], ya[:, h % 2, :],
                                mybir.AluOpType.subtract)

                # ---------------- Phase C: output projection -------------
                with tc.tile_pool(name="psO", bufs=4, space="PSUM") as psO:
                    for tt in range(T // 128):
                        b = tt // 4
                        tsl = slice((tt % 4) * 128, (tt % 4) * 128 + 128)
                        o_row = orw.tile([128, C], BF16, tag="orow")
                        for cb in range(C // 512):
                            cs = slice(cb * 512, (cb + 1) * 512)
                            o_ps = psO.tile([128, 512], F32, tag="o")
                            terms = [(YA[0][b], WPH[0]), (YA[1][b], WPH[1]),
                                     (YB[0][b], WPH[0]), (YB[1][b], WPH[1]),
                                     (YA[0][b], WPL[0]), (YA[1][b], WPL[1])]
                            for t_i, (yt, wt) in enumerate(terms):
                                nc.tensor.matmul(
                                    o_ps[:], yt[:, :, tsl], wt[:, :, cs],
                                    start=(t_i == 0), stop=(t_i == 5),
                                    perf_mode=DR)
                            # split psum->sbuf copies between Act and DVE
                            if cb % 2 == 0:
                                nc.scalar.mul(o_row[:, cs], o_ps[:],
                                              1.0 / WSCALE)
                            else:
                                nc.vector.tensor_scalar_mul(
                                    o_row[:, cs], o_ps[:], 1.0 / WSCALE)
                        nc.sync.dma_start(
                            d_out[tt * 128:(tt + 1) * 128, :], o_row[:])
    nc.compile()
    return nc


def _host_inputs(x, cos, sin, w_attn, w_proj):
    """Build per-core input maps (host-side shard/transpose/fp8 split)."""
    f = np.float32
    f8 = ml_dtypes.float8_e4m3fn

    def split8(a):
        hi = a.astype(f8)
        lo = (a - hi.astype(f)).astype(f8)
        return hi, lo

    xt = x.reshape(T, C).T.astype(f)                         # [C, T]
    xh, xl = split8(xt)

    def xlayout(a):
        # [C, T] -> [tb, p, n, c] with C = n*128+p, T = tb*512+c
        return np.ascontiguousarray(
            a.reshape(NC, 128, NT, 512).transpose(2, 1, 0, 3))

    xh, xl = xlayout(xh), xlayout(xl)
    cost = np.ascontiguousarray(cos.T.astype(f) / WSCALE)    # [HS, T]
    sgn = np.ones((HS, 1), f)
    sgn[:HS // 2] = -1.0
    sint = np.ascontiguousarray((sin.T * sgn).astype(f) / WSCALE)
    # rot(x)=P@x in [d,t] layout; matmul computes lhsT.T @ rhs -> lhsT = P.T
    P = np.zeros((HS, HS), f)
    for i in range(HS // 2):
        P[i, i + HS // 2] = 1.0
        P[i + HS // 2, i] = 1.0
    perm = np.ascontiguousarray(P.T)
    idn = np.eye(128, dtype=f).astype(ml_dtypes.bfloat16)
    onesb = np.ones((128, 128), ml_dtypes.bfloat16)
    onesr = np.ones((128, 128), f)
    # causal mask strip: within a diagonal 128x128 tile, q offset i vs key
    # partition p: keep iff i >= p (same strip for every diagonal offset r)
    pidx = np.arange(128)
    cols = np.arange(128)
    mask = (cols[None, :] >= pidx[:, None]).astype(f).astype(
        ml_dtypes.bfloat16)

    def wlayout(a):
        # [C, RG] -> [p, n, col]
        return np.ascontiguousarray(
            a.reshape(NC, 128, RG).transpose(1, 0, 2))

    maps = []
    for g in range(N_CORES):
        wat = (w_attn[g * RG:(g + 1) * RG, :].T.astype(f) * WSCALE)  # [C, RG]
        wh, wl = split8(wat)
        wh, wl = wlayout(wh), wlayout(wl)
        wpt = (w_proj[:, g * OG:(g + 1) * OG].T.astype(f) * WSCALE)  # [OG, C]
        wph, wpl = split8(wpt)
        maps.append({
            "xh": xh, "xl": xl, "wh": wh, "wl": wl,
            "wph": np.ascontiguousarray(wph),
            "wpl": np.ascontiguousarray(wpl),
            "cost": cost, "sint": sint, "mask": mask, "perm": perm,
            "idn": idn, "onesb": onesb, "onesr": onesr,
        })
    return maps


_PROGRAM = None


def kernel(x, cos, sin, w_attn, w_proj):
    global _PROGRAM
    if _PROGRAM is None:
        _PROGRAM = _build_program()
    maps = _host_inputs(np.asarray(x), np.asarray(cos), np.asarray(sin),
                        np.asarray(w_attn), np.asarray(w_proj))
    res = bass_utils.run_bass_kernel_spmd(_PROGRAM, maps, list(range(N_CORES)))
    out = np.zeros((T, C), np.float32)
    for g in range(N_CORES):
        out += np.asarray(res.results[g]["out"]).astype(np.float32)
    return out.reshape(1, T, C)


# revision 12
# speedup vs baseline: 1.3206x; 1.0258x over previous
"""Trainium2 Bass kernel for CausalSelfAttention (B=1, T=2048, C=4096,
32 heads / 8 query groups / head_size 128, full-dim RoPE, GQA).

Sharding: tensor-parallel over the 8 query groups. Core g owns w_attn rows
[g*768:(g+1)*768] (4 q heads + 1 k + 1 v) and w_proj columns
[g*512:(g+1)*512]; x is replicated. Each core returns a bf16 partial
projection output [2048, 4096]; the host sums the 8 partials.

Matmul precision strategy:
- QKV projection and output projection run as fp8e4m3 DoubleRow matmuls
  with a 3-term split (w = wh+wl, x = xh+xl; wh@xh + wh@xl + wl@xh), with
  weights pre-scaled by 64 on the host to avoid fp8 subnormals.
- Attention (QK^T, P@V) runs in bf16; softmax denominator via pair/quad
  summed P tiles (bf16 adds on DVE) + ones-matmuls into PSUM.
- All DMA transfers are batched (contiguous-per-partition host layouts):
  each dma_start costs ~625ns of serialized descriptor generation
  regardless of size, so few big transfers beat many small ones.
"""

import os
import sys

for _p in ("/opt/trn_rl_repo", "/root/.axon_site/_ro/trn_rl_repo"):
    if os.path.isdir(_p) and _p not in sys.path:
        sys.path.insert(0, _p)

import numpy as np
import ml_dtypes

import concourse.bass as bass
import concourse.mybir as mybir
import concourse.tile as tile
from concourse import bacc, bass_utils

N_CORES = 8
T = 2048
C = 4096
HS = 128
G = 8                      # query groups == cores
QPK = 4                    # q heads per group
NCOMP = QPK + 2            # q0..q3, k, v
RG = NCOMP * HS            # 768 w_attn rows per group
OG = QPK * HS              # 512 proj-input cols per group
NT = T // 512              # 4 blocks of 512 along t
NC = C // 128              # 32 contraction chunks
NM = NC // 2               # 16 chunk-pairs
NQ = 4                     # quarters (4 chunk-pairs each) per contraction
SCALE = 1.0 / np.sqrt(float(HS))
WSCALE = 64.0              # host pre-scale for w_attn / w_proj fp8

F32 = mybir.dt.float32
F32R = mybir.dt.float32r
BF16 = mybir.dt.bfloat16
FP8 = mybir.dt.float8e4
DR = mybir.MatmulPerfMode.DoubleRow


def _build_program():
    nc = bacc.Bacc(trn_type="TRN2", target_bir_lowering=False, debug=False,
                   num_devices=N_CORES)

    # x splits laid out [tb, p, n, c]; w splits laid out [p, n, col]
    d_xh = nc.dram_tensor("xh", [NT, 128, NC, 512], FP8,
                          kind="ExternalInput").ap()
    d_xl = nc.dram_tensor("xl", [NT, 128, NC, 512], FP8,
                          kind="ExternalInput").ap()
    d_wh = nc.dram_tensor("wh", [128, NC, RG], FP8, kind="ExternalInput").ap()
    d_wl = nc.dram_tensor("wl", [128, NC, RG], FP8, kind="ExternalInput").ap()
    d_wph = nc.dram_tensor("wph", [OG, C], FP8, kind="ExternalInput").ap()
    d_wpl = nc.dram_tensor("wpl", [OG, C], FP8, kind="ExternalInput").ap()
    d_cos = nc.dram_tensor("cost", [HS, T], F32R, kind="ExternalInput").ap()
    d_sin = nc.dram_tensor("sint", [HS, T], F32R, kind="ExternalInput").ap()
    d_mask = nc.dram_tensor("mask", [128, 128], BF16,
                            kind="ExternalInput").ap()
    d_perm = nc.dram_tensor("perm", [128, 128], F32R,
                            kind="ExternalInput").ap()
    d_idn = nc.dram_tensor("idn", [128, 128], BF16, kind="ExternalInput").ap()
    d_onesb = nc.dram_tensor("onesb", [128, 128], BF16,
                             kind="ExternalInput").ap()
    d_onesr = nc.dram_tensor("onesr", [128, 128], F32R,
                             kind="ExternalInput").ap()
    d_out = nc.dram_tensor("out", [T, C], BF16, kind="ExternalOutput").ap()

    with tile.TileContext(nc) as tc:
        with tc.tile_pool(name="glob", bufs=1) as glob:
            # roped q0..q3 / k in bf16, one tile per (comp, t-block)
            QQ = [[glob.tile([128, 512], BF16, name=f"qq{j}_{tb}",
                             tag=f"qq{j}_{tb}")
                   for tb in range(NT)] for j in range(5)]
            # V in [t, hs] layout (bf16), col u = t-chunk
            V = [glob.tile([128, 512], BF16, name=f"v{tb}", tag=f"v{tb}")
                 for tb in range(NT)]
            # proj inputs: fp8 hi/lo, head-paired, one tile per (pair, b)
            YA = [[glob.tile([128, 2, 512], FP8, name=f"ya{i}_{b}",
                             tag=f"ya{i}_{b}") for b in range(NT)]
                  for i in range(2)]
            YB = [[glob.tile([128, 2, 512], FP8, name=f"yb{i}_{b}",
                             tag=f"yb{i}_{b}") for b in range(NT)]
                  for i in range(2)]
            COS = glob.tile([128, T], F32R)
            SIN = glob.tile([128, T], F32R)
            MASK = glob.tile([128, 128], BF16)
            PERM = glob.tile([128, 128], F32R)
            IDN = glob.tile([128, 128], BF16)
            ONESB = glob.tile([128, 128], BF16)
            ONESR = glob.tile([128, 128], F32R)

            def load_consts():
                nc.sync.dma_start(COS[:], d_cos[:])
                nc.sync.dma_start(SIN[:], d_sin[:])
                nc.sync.dma_start(MASK[:], d_mask[:])
                nc.sync.dma_start(PERM[:], d_perm[:])
                nc.sync.dma_start(IDN[:], d_idn[:])
                nc.sync.dma_start(ONESB[:], d_onesb[:])
                nc.sync.dma_start(ONESR[:], d_onesr[:])

            # ---------------- Phase A: qkv projection + rope -------------
            with tc.tile_pool(name="wa", bufs=1) as wap, \
                 tc.tile_pool(name="xp", bufs=2) as xp, \
                 tc.tile_pool(name="tmpa", bufs=2) as tmpa, \
                 tc.tile_pool(name="psA", bufs=1, space="PSUM") as psA, \
                 tc.tile_pool(name="psR", bufs=2, space="PSUM") as psR:
                WH = [wap.tile([128, 8, RG], FP8, name=f"whq{q}",
                               tag=f"wh{q}") for q in range(NQ)]
                WL = [wap.tile([128, 8, RG], FP8, name=f"wlq{q}",
                               tag=f"wl{q}") for q in range(NQ)]

                for tb in range(NT):
                    ts = slice(tb * 512, (tb + 1) * 512)
                    qkv_ps = [psA.tile([128, 512], F32, tag=f"qkv{j}",
                                       name=f"qkv{j}")
                              for j in range(NCOMP)]
                    XH = [xp.tile([128, 8, 512], FP8, name=f"xhq{q}",
                                  tag=f"xh{q}") for q in range(NQ)]
                    XL = [xp.tile([128, 8, 512], FP8, name=f"xlq{q}",
                                  tag=f"xl{q}") for q in range(NQ)]
                    for q in range(NQ):
                        cs = slice(q * 8, (q + 1) * 8)
                        if tb == 0:
                            nc.sync.dma_start(WH[q][:], d_wh[:, cs, :])
                            nc.sync.dma_start(WL[q][:], d_wl[:, cs, :])
                        nc.sync.dma_start(XH[q][:], d_xh[tb, :, cs, :])
                        nc.sync.dma_start(XL[q][:], d_xl[tb, :, cs, :])
                        if tb == 0 and q == 0:
                            # consts go after the first compute quarter so
                            # the PE warmup isn't stuck behind them
                            load_consts()
                    # m-outer so PE consumption tracks the quarter DMAs
                    for m in range(NM):
                        q, u = m // 4, m % 4
                        ps = slice(2 * u, 2 * u + 2)
                        for j in range(NCOMP):
                            js = slice(j * HS, (j + 1) * HS)
                            for t_i, (wt, xt) in enumerate(
                                    ((WH, XH), (WH, XL), (WL, XH))):
                                nc.tensor.matmul(
                                    qkv_ps[j][:],
                                    wt[q][:, ps, js],
                                    xt[q][:, ps, :],
                                    start=(m == 0 and t_i == 0),
                                    stop=(m == NM - 1 and t_i == 2),
                                    perf_mode=DR)

                    for j in range(5):  # q0..q3, k get rope
                        raw = tmpa.tile([128, 512], F32R, tag="raw")
                        nc.scalar.copy(raw[:], qkv_ps[j][:])
                        rot = psR.tile([128, 512], F32, tag="rot")
                        nc.tensor.matmul(rot[:], PERM[:], raw[:],
                                         start=True, stop=True)
                        t1 = tmpa.tile([128, 512], F32R, tag="t1")
                        nc.vector.tensor_tensor(t1[:], raw[:], COS[:, ts],
                                                mybir.AluOpType.mult)
                        t2 = tmpa.tile([128, 512], F32R, tag="t2")
                        nc.vector.tensor_tensor(t2[:], rot[:], SIN[:, ts],
                                                mybir.AluOpType.mult)
                        nc.vector.tensor_tensor(QQ[j][tb][:], t1[:], t2[:],
                                                mybir.AluOpType.add)

                    # v: scale back by 1/64, transpose [hs, t] -> [t, hs]
                    vraw = tmpa.tile([128, 512], BF16, tag="vraw")
                    nc.scalar.mul(vraw[:], qkv_ps[5][:], 1.0 / WSCALE)
                    for u in range(4):
                        vt = psR.tile([128, 128], BF16, tag="rot")
                        nc.tensor.transpose(vt[:],
                                            vraw[:, u * 128:(u + 1) * 128],
                                            IDN[:])
                        nc.vector.tensor_copy(
                            V[tb][:, u * 128:(u + 1) * 128], vt[:])

            # ---------------- Phase B: causal attention ------------------
            with tc.tile_pool(name="wp", bufs=1) as wpp, \
                 tc.tile_pool(name="pwp", bufs=4) as pwp, \
                 tc.tile_pool(name="prp", bufs=3) as prp, \
                 tc.tile_pool(name="bcp", bufs=2) as bcp, \
                 tc.tile_pool(name="yfp", bufs=2) as yfp, \
                 tc.tile_pool(name="orw", bufs=2) as orw, \
                 tc.tile_pool(name="rcp", bufs=2) as rcp:
                WPH = [wpp.tile([128, 2, C], FP8, name=f"wph{i}",
                                tag=f"wph{i}") for i in range(2)]
                WPL = [wpp.tile([128, 2, C], FP8, name=f"wpl{i}",
                                tag=f"wpl{i}") for i in range(2)]
                for h in range(QPK):
                    nc.sync.dma_start(WPH[h // 2][:, h % 2, :],
                                      d_wph[h * 128:(h + 1) * 128, :])
                    nc.sync.dma_start(WPL[h // 2][:, h % 2, :],
                                      d_wpl[h * 128:(h + 1) * 128, :])

                with tc.tile_pool(name="psS", bufs=2, space="PSUM") as psS, \
                     tc.tile_pool(name="psY", bufs=1, space="PSUM") as psY, \
                     tc.tile_pool(name="psD", bufs=1, space="PSUM") as psD, \
                     tc.tile_pool(name="psO", bufs=2, space="PSUM") as psO:

                    def emit_b_block(b):
                        for h in range(QPK):
                            nkt = 4 * (b + 1)
                            y_ps = psY.tile([128, 512], F32, tag="y")
                            d_ps = psD.tile([1, 512], F32, tag="d")
                            first_d = [True]
                            quad = [None]

                            def d_mm(rhs_ap, off, last):
                                nc.tensor.matmul(
                                    d_ps[:, off:], ONESB[:, :1], rhs_ap,
                                    start=first_d[0], stop=last)
                                first_d[0] = False

                            for u in range(nkt // 2):
                                s_w = psS.tile([128, 1024], F32, tag="s")
                                p_w = pwp.tile([128, 1024], BF16, tag="p")
                                offs = []
                                for half in range(2):
                                    kt = 2 * u + half
                                    r = kt - 4 * b
                                    off = 0 if r < 0 else r * 128
                                    offs.append(off)
                                    co = half * 512
                                    nc.tensor.matmul(
                                        s_w[:, co + off:co + 512],
                                        QQ[4][kt // 4][:, (kt % 4) * 128:
                                                       (kt % 4 + 1) * 128],
                                        QQ[h][b][:, off:],
                                        start=True, stop=True)
                                full_pair = offs[0] == 0 and offs[1] == 0
                                if full_pair:
                                    nc.scalar.activation(
                                        p_w[:], s_w[:],
                                        mybir.ActivationFunctionType.Exp,
                                        scale=SCALE)
                                else:
                                    for half in range(2):
                                        co = half * 512 + offs[half]
                                        nc.scalar.activation(
                                            p_w[:, co:half * 512 + 512],
                                            s_w[:, co:half * 512 + 512],
                                            mybir.ActivationFunctionType.Exp,
                                            scale=SCALE)
                                for half in range(2):
                                    kt = 2 * u + half
                                    r = kt - 4 * b
                                    if r >= 0:  # diagonal: mask 128-col strip
                                        co = half * 512 + r * 128
                                        nc.vector.tensor_tensor(
                                            p_w[:, co:co + 128],
                                            p_w[:, co:co + 128],
                                            MASK[:],
                                            mybir.AluOpType.mult)
                                for half in range(2):
                                    kt = 2 * u + half
                                    off = offs[half]
                                    co = half * 512
                                    nc.tensor.matmul(
                                        y_ps[:, off:],
                                        V[kt // 4][:, (kt % 4) * 128:
                                                   (kt % 4 + 1) * 128],
                                        p_w[:, co + off:co + 512],
                                        start=(kt == 0), stop=(kt == nkt - 1))
                                # denominator: quad-sum full tiles on DVE,
                                # one ones-matmul per quad / diagonal half
                                if full_pair:
                                    pr = prp.tile([128, 512], BF16, tag="pr")
                                    nc.vector.tensor_tensor(
                                        pr[:], p_w[:, :512], p_w[:, 512:],
                                        mybir.AluOpType.add)
                                    if u % 2 == 0:
                                        quad[0] = pr
                                    else:
                                        q2 = prp.tile([128, 512], BF16,
                                                      tag="q2")
                                        nc.vector.tensor_tensor(
                                            q2[:], quad[0][:], pr[:],
                                            mybir.AluOpType.add)
                                        quad[0] = None
                                        d_mm(q2[:], 0, False)
                                else:
                                    if quad[0] is not None:
                                        d_mm(quad[0][:], 0, False)
                                        quad[0] = None
                                    for half in range(2):
                                        off = offs[half]
                                        co = half * 512
                                        d_mm(p_w[:, co + off:co + 512], off,
                                             (u == nkt // 2 - 1 and
                                              half == 1))
                            recip = rcp.tile([1, 512], F32R, tag="r")
                            with nc.allow_low_precision(
                                    reason="f32r is f32-width"):
                                nc.vector.reciprocal(recip[:], d_ps[:])
                            bc_ps = psD.tile([128, 512], F32, tag="d")
                            nc.tensor.matmul(bc_ps[:], ONESR[:1, :], recip[:],
                                             start=True, stop=True)
                            bc_sb = bcp.tile([128, 512], BF16, tag="bc")
                            nc.vector.tensor_copy(bc_sb[:], bc_ps[:])
                            yf = yfp.tile([128, 512], F32R, tag="yf")
                            nc.vector.tensor_tensor(yf[:], y_ps[:], bc_sb[:],
                                                    mybir.AluOpType.mult)
                            ya = YA[h // 2][b]
                            yb = YB[h // 2][b]
                            nc.vector.tensor_copy(ya[:, h % 2, :], yf[:])
                            nc.vector.tensor_tensor(
                                yb[:, h % 2, :], yf[:], ya[:, h % 2, :],
                                mybir.AluOpType.subtract)

                    # ------------- Phase C: output projection ------------
                    def emit_c_block(b):
                        for tt in range(4 * b, 4 * b + 4):
                            tsl = slice((tt % 4) * 128, (tt % 4) * 128 + 128)
                            o_row = orw.tile([128, C], BF16, tag="orow")
                            for cb in range(C // 512):
                                cs = slice(cb * 512, (cb + 1) * 512)
                                o_ps = psO.tile([128, 512], F32, tag="o")
                                terms = [(YA[0][b], WPH[0]),
                                         (YA[1][b], WPH[1]),
                                         (YB[0][b], WPH[0]),
                                         (YB[1][b], WPH[1]),
                                         (YA[0][b], WPL[0]),
                                         (YA[1][b], WPL[1])]
                                for t_i, (yt, wt) in enumerate(terms):
                                    nc.tensor.matmul(
                                        o_ps[:], yt[:, :, tsl], wt[:, :, cs],
                                        start=(t_i == 0), stop=(t_i == 5),
                                        perf_mode=DR)
                                # split psum->sbuf copies across Act and DVE
                                if cb % 2 == 0:
                                    nc.scalar.mul(o_row[:, cs], o_ps[:],
                                                  1.0 / WSCALE)
                                else:
                                    nc.vector.tensor_scalar_mul(
                                        o_row[:, cs], o_ps[:], 1.0 / WSCALE)
                            for dh in range(2):
                                hs_ = slice(dh * 2048, (dh + 1) * 2048)
                                nc.sync.dma_start(
                                    d_out[tt * 128:(tt + 1) * 128, hs_],
                                    o_row[:, hs_])

                    # interleave: C(b-1) emitted after B(b) so its matmuls
                    # can fill PE bubbles while B waits on exp
                    emit_b_block(0)
                    emit_b_block(1)
                    emit_c_block(0)
                    emit_b_block(2)
                    emit_c_block(1)
                    emit_b_block(3)
                    emit_c_block(2)
                    emit_c_block(3)
    nc.compile()
    return nc


def _host_inputs(x, cos, sin, w_attn, w_proj):
    """Build per-core input maps (host-side shard/transpose/fp8 split)."""
    f = np.float32
    f8 = ml_dtypes.float8_e4m3fn

    def split8(a):
        hi = a.astype(f8)
        lo = (a - hi.astype(f)).astype(f8)
        return hi, lo

    xt = x.reshape(T, C).T.astype(f)                         # [C, T]
    xh, xl = split8(xt)

    def xlayout(a):
        # [C, T] -> [tb, p, n, c] with C = n*128+p, T = tb*512+c
        return np.ascontiguousarray(
            a.reshape(NC, 128, NT, 512).transpose(2, 1, 0, 3))

    xh, xl = xlayout(xh), xlayout(xl)
    cost = np.ascontiguousarray(cos.T.astype(f) / WSCALE)    # [HS, T]
    sgn = np.ones((HS, 1), f)
    sgn[:HS // 2] = -1.0
    sint = np.ascontiguousarray((sin.T * sgn).astype(f) / WSCALE)
    # rot(x)=P@x in [d,t] layout; matmul computes lhsT.T @ rhs -> lhsT = P.T
    P = np.zeros((HS, HS), f)
    for i in range(HS // 2):
        P[i, i + HS // 2] = 1.0
        P[i + HS // 2, i] = 1.0
    perm = np.ascontiguousarray(P.T)
    idn = np.eye(128, dtype=f).astype(ml_dtypes.bfloat16)
    onesb = np.ones((128, 128), ml_dtypes.bfloat16)
    onesr = np.ones((128, 128), f)
    # causal mask strip: within a diagonal 128x128 tile, q offset i vs key
    # partition p: keep iff i >= p (same strip for every diagonal offset r)
    pidx = np.arange(128)
    cols = np.arange(128)
    mask = (cols[None, :] >= pidx[:, None]).astype(f).astype(
        ml_dtypes.bfloat16)

    def wlayout(a):
        # [C, RG] -> [p, n, col]
        return np.ascontiguousarray(
            a.reshape(NC, 128, RG).transpose(1, 0, 2))

    maps = []
    for g in range(N_CORES):
        wat = (w_attn[g * RG:(g + 1) * RG, :].T.astype(f) * WSCALE)  # [C, RG]
        wh, wl = split8(wat)
        wh, wl = wlayout(wh), wlayout(wl)
        wpt = (w_proj[:, g * OG:(g + 1) * OG].T.astype(f) * WSCALE)  # [OG, C]
        wph, wpl = split8(wpt)
        maps.append({
            "xh": xh, "xl": xl, "wh": wh, "wl": wl,
            "wph": np.ascontiguousarray(wph),
            "wpl": np.ascontiguousarray(wpl),
            "cost": cost, "sint": sint, "mask": mask, "perm": perm,
            "idn": idn, "onesb": onesb, "onesr": onesr,
        })
    return maps


_PROGRAM = None


def kernel(x, cos, sin, w_attn, w_proj):
    global _PROGRAM
    if _PROGRAM is None:
        _PROGRAM = _build_program()
    maps = _host_inputs(np.asarray(x), np.asarray(cos), np.asarray(sin),
                        np.asarray(w_attn), np.asarray(w_proj))
    res = bass_utils.run_bass_kernel_spmd(_PROGRAM, maps, list(range(N_CORES)))
    out = np.zeros((T, C), np.float32)
    for g in range(N_CORES):
        out += np.asarray(res.results[g]["out"]).astype(np.float32)
    return out.reshape(1, T, C)


# revision 17
# speedup vs baseline: 1.3761x; 1.0420x over previous
"""Trainium2 Bass kernel for CausalSelfAttention (B=1, T=2048, C=4096,
32 heads / 8 query groups / head_size 128, full-dim RoPE, GQA).

Sharding: tensor-parallel over the 8 query groups. Core g owns w_attn rows
[g*768:(g+1)*768] (4 q heads + 1 k + 1 v) and w_proj columns
[g*512:(g+1)*512]; x is replicated. Each core returns a bf16 partial
projection output [2048, 4096]; the host sums the 8 partials.

Matmul precision strategy (rel err ~3.5e-3 vs fp32 reference):
- QKV projection and output projection run as fp8e4m3 DoubleRow matmuls
  with a 3-term split (w = wh+wl, x = xh+xl; wh@xh + wh@xl + wl@xh), with
  weights pre-scaled by 64 on the host to avoid fp8 subnormals. The splits
  are computed host-side; only the attention output y is split on-device.
- Attention (QK^T, P@V) runs in bf16 with f32 PSUM; softmax denominator
  via pair/quad-summed P tiles (bf16 adds on DVE) + ones-matmuls.

Schedule/engine layout:
- All DMA transfers are batched (contiguous-per-partition host layouts):
  each dma_start costs ~625ns of serialized descriptor generation
  regardless of size, so few big transfers beat many small ones. Constants
  load after the first weight/x quarters so PE warmup isn't blocked.
- Phase C (proj) chunks are emitted interleaved between attention blocks
  so their DoubleRow matmuls fill PE bubbles while softmax exp (Act
  engine) is the per-block critical path; PSUM->SBUF output copies are
  split across Act and DVE.
"""

import os
import sys

for _p in ("/opt/trn_rl_repo", "/root/.axon_site/_ro/trn_rl_repo"):
    if os.path.isdir(_p) and _p not in sys.path:
        sys.path.insert(0, _p)

import numpy as np
import ml_dtypes

import concourse.bass as bass
import concourse.mybir as mybir
import concourse.tile as tile
from concourse import bacc, bass_utils

N_CORES = 8
T = 2048
C = 4096
HS = 128
G = 8                      # query groups == cores
QPK = 4                    # q heads per group
NCOMP = QPK + 2            # q0..q3, k, v
RG = NCOMP * HS            # 768 w_attn rows per group
OG = QPK * HS              # 512 proj-input cols per group
NT = T // 512              # 4 blocks of 512 along t
NC = C // 128              # 32 contraction chunks
NM = NC // 2               # 16 chunk-pairs
NQ = 4                     # quarters (4 chunk-pairs each) per contraction
SCALE = 1.0 / np.sqrt(float(HS))
WSCALE = 64.0              # host pre-scale for w_attn / w_proj fp8

F32 = mybir.dt.float32
F32R = mybir.dt.float32r
BF16 = mybir.dt.bfloat16
FP8 = mybir.dt.float8e4
DR = mybir.MatmulPerfMode.DoubleRow


def _build_program():
    nc = bacc.Bacc(trn_type="TRN2", target_bir_lowering=False, debug=False,
                   num_devices=N_CORES)

    # x splits laid out [tb, p, n, c]; w splits laid out [p, n, col]
    d_xh = nc.dram_tensor("xh", [NT, 128, NC, 512], FP8,
                          kind="ExternalInput").ap()
    d_xl = nc.dram_tensor("xl", [NT, 128, NC, 512], FP8,
                          kind="ExternalInput").ap()
    d_wh = nc.dram_tensor("wh", [128, NC, RG], FP8, kind="ExternalInput").ap()
    d_wl = nc.dram_tensor("wl", [128, NC, RG], FP8, kind="ExternalInput").ap()
    d_wph = nc.dram_tensor("wph", [OG, C], FP8, kind="ExternalInput").ap()
    d_wpl = nc.dram_tensor("wpl", [OG, C], FP8, kind="ExternalInput").ap()
    d_cos = nc.dram_tensor("cost", [HS, T], F32R, kind="ExternalInput").ap()
    d_sin = nc.dram_tensor("sint", [HS, T], F32R, kind="ExternalInput").ap()
    d_mask = nc.dram_tensor("mask", [128, 128], BF16,
                            kind="ExternalInput").ap()
    d_perm = nc.dram_tensor("perm", [128, 128], F32R,
                            kind="ExternalInput").ap()
    d_idn = nc.dram_tensor("idn", [128, 128], BF16, kind="ExternalInput").ap()
    d_onesb = nc.dram_tensor("onesb", [128, 128], BF16,
                             kind="ExternalInput").ap()
    d_onesr = nc.dram_tensor("onesr", [128, 128], F32R,
                             kind="ExternalInput").ap()
    d_out = nc.dram_tensor("out", [T, C], BF16, kind="ExternalOutput").ap()

    with tile.TileContext(nc) as tc:
        with tc.tile_pool(name="glob", bufs=1) as glob:
            # roped q0..q3 / k in bf16, one tile per (comp, t-block)
            QQ = [[glob.tile([128, 512], BF16, name=f"qq{j}_{tb}",
                             tag=f"qq{j}_{tb}")
                   for tb in range(NT)] for j in range(5)]
            # V in [t, hs] layout (bf16), col u = t-chunk
            V = [glob.tile([128, 512], BF16, name=f"v{tb}", tag=f"v{tb}")
                 for tb in range(NT)]
            # proj inputs: fp8 hi/lo, head-paired, one tile per (pair, b)
            YA = [[glob.tile([128, 2, 512], FP8, name=f"ya{i}_{b}",
                             tag=f"ya{i}_{b}") for b in range(NT)]
                  for i in range(2)]
            YB = [[glob.tile([128, 2, 512], FP8, name=f"yb{i}_{b}",
                             tag=f"yb{i}_{b}") for b in range(NT)]
                  for i in range(2)]
            COS = glob.tile([128, T], F32R)
            SIN = glob.tile([128, T], F32R)
            MASK = glob.tile([128, 128], BF16)
            PERM = glob.tile([128, 128], F32R)
            IDN = glob.tile([128, 128], BF16)
            ONESB = glob.tile([128, 128], BF16)
            ONESR = glob.tile([128, 128], F32R)

            def load_consts():
                nc.sync.dma_start(PERM[:], d_perm[:])
                nc.sync.dma_start(IDN[:], d_idn[:])

            def load_consts2():
                nc.sync.dma_start(COS[:], d_cos[:])
                nc.sync.dma_start(SIN[:], d_sin[:])
                nc.sync.dma_start(MASK[:], d_mask[:])
                nc.sync.dma_start(ONESB[:], d_onesb[:])
                nc.sync.dma_start(ONESR[:], d_onesr[:])

            # ---------------- Phase A: qkv projection + rope -------------
            with tc.tile_pool(name="wa", bufs=1) as wap, \
                 tc.tile_pool(name="xp", bufs=2) as xp, \
                 tc.tile_pool(name="tmpa", bufs=2) as tmpa, \
                 tc.tile_pool(name="psA", bufs=1, space="PSUM") as psA, \
                 tc.tile_pool(name="psR", bufs=2, space="PSUM") as psR:
                WH = [wap.tile([128, 8, RG], FP8, name=f"whq{q}",
                               tag=f"wh{q}") for q in range(NQ)]
                WL = [wap.tile([128, 8, RG], FP8, name=f"wlq{q}",
                               tag=f"wl{q}") for q in range(NQ)]

                for tb in range(NT):
                    ts = slice(tb * 512, (tb + 1) * 512)
                    qkv_ps = [psA.tile([128, 512], F32, tag=f"qkv{j}",
                                       name=f"qkv{j}")
                              for j in range(NCOMP)]
                    XH = [xp.tile([128, 8, 512], FP8, name=f"xhq{q}",
                                  tag=f"xh{q}") for q in range(NQ)]
                    XL = [xp.tile([128, 8, 512], FP8, name=f"xlq{q}",
                                  tag=f"xl{q}") for q in range(NQ)]
                    for q in range(NQ):
                        cs = slice(q * 8, (q + 1) * 8)
                        if tb == 0 and q == 0:
                            # first quarter in halves: earliest possible start
                            for hh in range(2):
                                c4 = slice(4 * hh, 4 * hh + 4)
                                nc.sync.dma_start(WH[0][:, c4, :],
                                                  d_wh[:, c4, :])
                                nc.sync.dma_start(XH[0][:, c4, :],
                                                  d_xh[0, :, c4, :])
                            nc.sync.dma_start(WL[0][:], d_wl[:, cs, :])
                            nc.sync.dma_start(XL[0][:], d_xl[0, :, cs, :])
                        else:
                            if tb == 0:
                                nc.sync.dma_start(WH[q][:], d_wh[:, cs, :])
                            nc.sync.dma_start(XH[q][:], d_xh[tb, :, cs, :])
                            if tb == 0:
                                nc.sync.dma_start(WL[q][:], d_wl[:, cs, :])
                            nc.sync.dma_start(XL[q][:], d_xl[tb, :, cs, :])
                        if tb == 0 and q == 0:
                            # consts go after the first compute quarter so
                            # the PE warmup isn't stuck behind them
                            load_consts()
                        if tb == 0 and q == NQ - 1:
                            load_consts2()
                    # quarter-outer, term-grouped: term a's inputs land
                    # first, so its matmuls run while b/c inputs stream in
                    for q in range(NQ):
                        for t_i, (wt, xt) in enumerate(
                                ((WH, XH), (WH, XL), (WL, XH))):
                            for u in range(4):
                                ps = slice(2 * u, 2 * u + 2)
                                for j in range(NCOMP):
                                    js = slice(j * HS, (j + 1) * HS)
                                    nc.tensor.matmul(
                                        qkv_ps[j][:],
                                        wt[q][:, ps, js],
                                        xt[q][:, ps, :],
                                        start=(q == 0 and t_i == 0 and
                                               u == 0),
                                        stop=(q == NQ - 1 and t_i == 2 and
                                              u == 3),
                                        perf_mode=DR)

                    for j in range(5):  # q0..q3, k get rope
                        raw = tmpa.tile([128, 512], F32R, tag="raw")
                        nc.scalar.copy(raw[:], qkv_ps[j][:])
                        rot = psR.tile([128, 512], F32, tag="rot")
                        nc.tensor.matmul(rot[:], PERM[:], raw[:],
                                         start=True, stop=True)
                        t1 = tmpa.tile([128, 512], F32R, tag="t1")
                        nc.vector.tensor_tensor(t1[:], raw[:], COS[:, ts],
                                                mybir.AluOpType.mult)
                        t2 = tmpa.tile([128, 512], F32R, tag="t2")
                        nc.vector.tensor_tensor(t2[:], rot[:], SIN[:, ts],
                                                mybir.AluOpType.mult)
                        nc.vector.tensor_tensor(QQ[j][tb][:], t1[:], t2[:],
                                                mybir.AluOpType.add)

                    # v: scale back by 1/64, transpose [hs, t] -> [t, hs]
                    vraw = tmpa.tile([128, 512], BF16, tag="vraw")
                    nc.scalar.mul(vraw[:], qkv_ps[5][:], 1.0 / WSCALE)
                    for u in range(4):
                        vt = psR.tile([128, 128], BF16, tag="rot")
                        nc.tensor.transpose(vt[:],
                                            vraw[:, u * 128:(u + 1) * 128],
                                            IDN[:])
                        nc.vector.tensor_copy(
                            V[tb][:, u * 128:(u + 1) * 128], vt[:])

            # ---------------- Phase B: causal attention ------------------
            with tc.tile_pool(name="wp", bufs=1) as wpp, \
                 tc.tile_pool(name="pwp", bufs=4) as pwp, \
                 tc.tile_pool(name="prp", bufs=3) as prp, \
                 tc.tile_pool(name="bcp", bufs=2) as bcp, \
                 tc.tile_pool(name="yfp", bufs=2) as yfp, \
                 tc.tile_pool(name="orw", bufs=2) as orw, \
                 tc.tile_pool(name="rcp", bufs=2) as rcp:
                WPH = [wpp.tile([128, 2, C], FP8, name=f"wph{i}",
                                tag=f"wph{i}") for i in range(2)]
                WPL = [wpp.tile([128, 2, C], FP8, name=f"wpl{i}",
                                tag=f"wpl{i}") for i in range(2)]
                for h in range(QPK):
                    nc.sync.dma_start(WPH[h // 2][:, h % 2, :],
                                      d_wph[h * 128:(h + 1) * 128, :])
                    nc.sync.dma_start(WPL[h // 2][:, h % 2, :],
                                      d_wpl[h * 128:(h + 1) * 128, :])

                with tc.tile_pool(name="psS", bufs=2, space="PSUM") as psS, \
                     tc.tile_pool(name="psY", bufs=1, space="PSUM") as psY, \
                     tc.tile_pool(name="psD", bufs=1, space="PSUM") as psD, \
                     tc.tile_pool(name="psO", bufs=2, space="PSUM") as psO:

                    def emit_b_block(b, cwork=None):
                        for h in range(QPK):
                            if cwork is not None:
                                emit_c_tt(cwork[0] * 4 + h)
                            nkt = 4 * (b + 1)
                            y_ps = psY.tile([128, 512], F32, tag="y")
                            d_ps = psD.tile([1, 512], F32, tag="d")
                            first_d = [True]
                            quad = [None]

                            def d_mm(rhs_ap, off, last):
                                nc.tensor.matmul(
                                    d_ps[:, off:], ONESB[:, :1], rhs_ap,
                                    start=first_d[0], stop=last)
                                first_d[0] = False

                            for u in range(nkt // 2):
                                s_w = psS.tile([128, 1024], F32, tag="s")
                                p_w = pwp.tile([128, 1024], BF16, tag="p")
                                offs = []
                                for half in range(2):
                                    kt = 2 * u + half
                                    r = kt - 4 * b
                                    off = 0 if r < 0 else r * 128
                                    offs.append(off)
                                    co = half * 512
                                    nc.tensor.matmul(
                                        s_w[:, co + off:co + 512],
                                        QQ[4][kt // 4][:, (kt % 4) * 128:
                                                       (kt % 4 + 1) * 128],
                                        QQ[h][b][:, off:],
                                        start=True, stop=True)
                                full_pair = offs[0] == 0 and offs[1] == 0
                                if full_pair:
                                    nc.scalar.activation(
                                        p_w[:], s_w[:],
                                        mybir.ActivationFunctionType.Exp,
                                        scale=SCALE)
                                else:
                                    for half in range(2):
                                        co = half * 512 + offs[half]
                                        nc.scalar.activation(
                                            p_w[:, co:half * 512 + 512],
                                            s_w[:, co:half * 512 + 512],
                                            mybir.ActivationFunctionType.Exp,
                                            scale=SCALE)
                                for half in range(2):
                                    kt = 2 * u + half
                                    r = kt - 4 * b
                                    if r >= 0:  # diagonal: mask 128-col strip
                                        co = half * 512 + r * 128
                                        nc.vector.tensor_tensor(
                                            p_w[:, co:co + 128],
                                            p_w[:, co:co + 128],
                                            MASK[:],
                                            mybir.AluOpType.mult)
                                for half in range(2):
                                    kt = 2 * u + half
                                    off = offs[half]
                                    co = half * 512
                                    nc.tensor.matmul(
                                        y_ps[:, off:],
                                        V[kt // 4][:, (kt % 4) * 128:
                                                   (kt % 4 + 1) * 128],
                                        p_w[:, co + off:co + 512],
                                        start=(kt == 0), stop=(kt == nkt - 1))
                                # denominator: quad-sum full tiles on DVE,
                                # one ones-matmul per quad / diagonal half
                                if full_pair:
                                    pr = prp.tile([128, 512], BF16, tag="pr")
                                    nc.vector.tensor_tensor(
                                        pr[:], p_w[:, :512], p_w[:, 512:],
                                        mybir.AluOpType.add)
                                    if u % 2 == 0:
                                        quad[0] = pr
                                    else:
                                        q2 = prp.tile([128, 512], BF16,
                                                      tag="q2")
                                        nc.vector.tensor_tensor(
                                            q2[:], quad[0][:], pr[:],
                                            mybir.AluOpType.add)
                                        quad[0] = None
                                        d_mm(q2[:], 0, False)
                                else:
                                    if quad[0] is not None:
                                        d_mm(quad[0][:], 0, False)
                                        quad[0] = None
                                    for half in range(2):
                                        off = offs[half]
                                        co = half * 512
                                        d_mm(p_w[:, co + off:co + 512], off,
                                             (u == nkt // 2 - 1 and
                                              half == 1))
                            recip = rcp.tile([1, 512], F32R, tag="r")
                            with nc.allow_low_precision(
                                    reason="f32r is f32-width"):
                                nc.vector.reciprocal(recip[:], d_ps[:])
                            bc_ps = psD.tile([128, 512], F32, tag="d")
                            nc.tensor.matmul(bc_ps[:], ONESR[:1, :], recip[:],
                                             start=True, stop=True)
                            bc_sb = bcp.tile([128, 512], BF16, tag="bc")
                            nc.vector.tensor_copy(bc_sb[:], bc_ps[:])
                            yf = yfp.tile([128, 512], F32R, tag="yf")
                            nc.vector.tensor_tensor(yf[:], y_ps[:], bc_sb[:],
                                                    mybir.AluOpType.mult)
                            ya = YA[h // 2][b]
                            yb = YB[h // 2][b]
                            nc.scalar.copy(ya[:, h % 2, :], yf[:])
                            nc.vector.tensor_tensor(
                                yb[:, h % 2, :], yf[:], ya[:, h % 2, :],
                                mybir.AluOpType.subtract)

                    # ------------- Phase C: output projection ------------
                    def emit_c_tt(tt):
                            b = tt // 4
                            tsl = slice((tt % 4) * 128, (tt % 4) * 128 + 128)
                            o_row = orw.tile([128, C], BF16, tag="orow")
                            for cb in range(C // 512):
                                cs = slice(cb * 512, (cb + 1) * 512)
                                o_ps = psO.tile([128, 512], F32, tag="o")
                                terms = [(YA[0][b], WPH[0]),
                                         (YA[1][b], WPH[1]),
                                         (YB[0][b], WPH[0]),
                                         (YB[1][b], WPH[1]),
                                         (YA[0][b], WPL[0]),
                                         (YA[1][b], WPL[1])]
                                for t_i, (yt, wt) in enumerate(terms):
                                    nc.tensor.matmul(
                                        o_ps[:], yt[:, :, tsl], wt[:, :, cs],
                                        start=(t_i == 0), stop=(t_i == 5),
                                        perf_mode=DR)
                                # split psum->sbuf copies across Act and DVE
                                if cb % 2 == 0:
                                    nc.scalar.mul(o_row[:, cs], o_ps[:],
                                                  1.0 / WSCALE)
                                else:
                                    nc.vector.tensor_scalar_mul(
                                        o_row[:, cs], o_ps[:], 1.0 / WSCALE)
                            for dh in range(2):
                                hs_ = slice(dh * 2048, (dh + 1) * 2048)
                                nc.sync.dma_start(
                                    d_out[tt * 128:(tt + 1) * 128, hs_],
                                    o_row[:, hs_])

                    # interleave: C(b-1) emitted after B(b) so its matmuls
                    # can fill PE bubbles while B waits on exp
                    emit_b_block(0)
                    emit_b_block(1)
                    for tt in range(0, 4):
                        emit_c_tt(tt)
                    emit_b_block(2)
                    for tt in range(4, 8):
                        emit_c_tt(tt)
                    emit_b_block(3)
                    for tt in range(8, 12):
                        emit_c_tt(tt)

                with tc.tile_pool(name="psO2", bufs=4, space="PSUM") as psO2:
                    for tt in range(12, 16):
                        b = tt // 4
                        tsl = slice((tt % 4) * 128, (tt % 4) * 128 + 128)
                        o_row = orw.tile([128, C], BF16, tag="orow")
                        for cb in range(C // 512):
                            cs = slice(cb * 512, (cb + 1) * 512)
                            o_ps = psO2.tile([128, 512], F32, tag="o")
                            terms = [(YA[0][b], WPH[0]),
                                     (YA[1][b], WPH[1]),
                                     (YB[0][b], WPH[0]),
                                     (YB[1][b], WPH[1]),
                                     (YA[0][b], WPL[0]),
                                     (YA[1][b], WPL[1])]
                            for t_i, (yt, wt) in enumerate(terms):
                                nc.tensor.matmul(
                                    o_ps[:], yt[:, :, tsl], wt[:, :, cs],
                                    start=(t_i == 0), stop=(t_i == 5),
                                    perf_mode=DR)
                            if cb % 2 == 0:
                                nc.scalar.mul(o_row[:, cs], o_ps[:],
                                              1.0 / WSCALE)
                            else:
                                nc.vector.tensor_scalar_mul(
                                    o_row[:, cs], o_ps[:], 1.0 / WSCALE)
                        for dh in range(4):
                            hs_ = slice(dh * 1024, (dh + 1) * 1024)
                            nc.sync.dma_start(
                                d_out[tt * 128:(tt + 1) * 128, hs_],
                                o_row[:, hs_])
    nc.compile()
    return nc


def _host_inputs(x, cos, sin, w_attn, w_proj):
    """Build per-core input maps (host-side shard/transpose/fp8 split)."""
    f = np.float32
    f8 = ml_dtypes.float8_e4m3fn

    def split8(a):
        hi = a.astype(f8)
        lo = (a - hi.astype(f)).astype(f8)
        return hi, lo

    xt = x.reshape(T, C).T.astype(f)                         # [C, T]
    xh, xl = split8(xt)

    def xlayout(a):
        # [C, T] -> [tb, p, n, c] with C = n*128+p, T = tb*512+c
        return np.ascontiguousarray(
            a.reshape(NC, 128, NT, 512).transpose(2, 1, 0, 3))

    xh, xl = xlayout(xh), xlayout(xl)
    cost = np.ascontiguousarray(cos.T.astype(f) / WSCALE)    # [HS, T]
    sgn = np.ones((HS, 1), f)
    sgn[:HS // 2] = -1.0
    sint = np.ascontiguousarray((sin.T * sgn).astype(f) / WSCALE)
    # rot(x)=P@x in [d,t] layout; matmul computes lhsT.T @ rhs -> lhsT = P.T
    P = np.zeros((HS, HS), f)
    for i in range(HS // 2):
        P[i, i + HS // 2] = 1.0
        P[i + HS // 2, i] = 1.0
    perm = np.ascontiguousarray(P.T)
    idn = np.eye(128, dtype=f).astype(ml_dtypes.bfloat16)
    onesb = np.ones((128, 128), ml_dtypes.bfloat16)
    onesr = np.ones((128, 128), f)
    # causal mask strip: within a diagonal 128x128 tile, q offset i vs key
    # partition p: keep iff i >= p (same strip for every diagonal offset r)
    pidx = np.arange(128)
    cols = np.arange(128)
    mask = (cols[None, :] >= pidx[:, None]).astype(f).astype(
        ml_dtypes.bfloat16)

    def wlayout(a):
        # [C, RG] -> [p, n, col]
        return np.ascontiguousarray(
            a.reshape(NC, 128, RG).transpose(1, 0, 2))

    maps = []
    for g in range(N_CORES):
        wat = (w_attn[g * RG:(g + 1) * RG, :].T.astype(f) * WSCALE)  # [C, RG]
        wh, wl = split8(wat)
        wh, wl = wlayout(wh), wlayout(wl)
        wpt = (w_proj[:, g * OG:(g + 1) * OG].T.astype(f) * WSCALE)  # [OG, C]
        wph, wpl = split8(wpt)
        maps.append({
            "xh": xh, "xl": xl, "wh": wh, "wl": wl,
            "wph": np.ascontiguousarray(wph),
            "wpl": np.ascontiguousarray(wpl),
            "cost": cost, "sint": sint, "mask": mask, "perm": perm,
            "idn": idn, "onesb": onesb, "onesr": onesr,
        })
    return maps


_PROGRAM = None


def kernel(x, cos, sin, w_attn, w_proj):
    global _PROGRAM
    if _PROGRAM is None:
        _PROGRAM = _build_program()
    maps = _host_inputs(np.asarray(x), np.asarray(cos), np.asarray(sin),
                        np.asarray(w_attn), np.asarray(w_proj))
    res = bass_utils.run_bass_kernel_spmd(_PROGRAM, maps, list(range(N_CORES)))
    out = np.zeros((T, C), np.float32)
    for g in range(N_CORES):
        out += np.asarray(res.results[g]["out"]).astype(np.float32)
    return out.reshape(1, T, C)



# revision 24
# speedup vs baseline: 1.3805x; 1.0032x over previous
"""Trainium2 Bass kernel for CausalSelfAttention (B=1, T=2048, C=4096,
32 heads / 8 query groups / head_size 128, full-dim RoPE, GQA).

Sharding: tensor-parallel over the 8 query groups. Core g owns w_attn rows
[g*768:(g+1)*768] (4 q heads + 1 k + 1 v) and w_proj columns
[g*512:(g+1)*512]; x is replicated. Each core returns a bf16 partial
projection output [2048, 4096]; the host sums the 8 partials.

Matmul precision strategy (rel err ~3.5e-3 vs fp32 reference):
- QKV projection and output projection run as fp8e4m3 DoubleRow matmuls
  with a 3-term split (w = wh+wl, x = xh+xl; wh@xh + wh@xl + wl@xh), with
  weights pre-scaled by 64 on the host to avoid fp8 subnormals. The splits
  are computed host-side; only the attention output y is split on-device.
- Attention (QK^T, P@V) runs in bf16 with f32 PSUM; softmax denominator
  via pair/quad-summed P tiles (bf16 adds on DVE) + ones-matmuls.

Schedule/engine layout:
- All DMA transfers are batched (contiguous-per-partition host layouts):
  each dma_start costs ~625ns of serialized descriptor generation
  regardless of size, so few big transfers beat many small ones. Constants
  load after the first weight/x quarters so PE warmup isn't blocked.
- Phase C (proj) chunks are emitted interleaved between attention blocks
  so their DoubleRow matmuls fill PE bubbles while softmax exp (Act
  engine) is the per-block critical path; PSUM->SBUF output copies are
  split across Act and DVE.
"""

import os
import sys

for _p in ("/opt/trn_rl_repo", "/root/.axon_site/_ro/trn_rl_repo"):
    if os.path.isdir(_p) and _p not in sys.path:
        sys.path.insert(0, _p)

import numpy as np
import ml_dtypes

import concourse.bass as bass
import concourse.mybir as mybir
import concourse.tile as tile
from concourse import bacc, bass_utils

N_CORES = 8
T = 2048
C = 4096
HS = 128
G = 8                      # query groups == cores
QPK = 4                    # q heads per group
NCOMP = QPK + 2            # q0..q3, k, v
RG = NCOMP * HS            # 768 w_attn rows per group
OG = QPK * HS              # 512 proj-input cols per group
NT = T // 512              # 4 blocks of 512 along t
NC = C // 128              # 32 contraction chunks
NM = NC // 2               # 16 chunk-pairs
NQ = 4                     # quarters (4 chunk-pairs each) per contraction
SCALE = 1.0 / np.sqrt(float(HS))
WSCALE = 64.0              # host pre-scale for w_attn / w_proj fp8

F32 = mybir.dt.float32
F32R = mybir.dt.float32r
BF16 = mybir.dt.bfloat16
FP8 = mybir.dt.float8e4
DR = mybir.MatmulPerfMode.DoubleRow


def _build_program():
    nc = bacc.Bacc(trn_type="TRN2", target_bir_lowering=False, debug=False,
                   num_devices=N_CORES)

    # x splits laid out [tb, p, n, c]; w splits laid out [p, n, col]
    d_xh = nc.dram_tensor("xh", [NT, 128, NC, 512], FP8,
                          kind="ExternalInput").ap()
    d_xl = nc.dram_tensor("xl", [NT, 128, NC, 512], FP8,
                          kind="ExternalInput").ap()
    d_wh = nc.dram_tensor("wh", [128, NC, RG], FP8, kind="ExternalInput").ap()
    d_wl = nc.dram_tensor("wl", [128, NC, RG], FP8, kind="ExternalInput").ap()
    d_wph = nc.dram_tensor("wph", [OG, C], FP8, kind="ExternalInput").ap()
    d_wpl = nc.dram_tensor("wpl", [OG, C], FP8, kind="ExternalInput").ap()
    d_cos = nc.dram_tensor("cost", [HS, T], F32R, kind="ExternalInput").ap()
    d_sin = nc.dram_tensor("sint", [HS, T], F32R, kind="ExternalInput").ap()
    d_mask = nc.dram_tensor("mask", [128, 128], BF16,
                            kind="ExternalInput").ap()
    d_perm = nc.dram_tensor("perm", [128, 128], F32R,
                            kind="ExternalInput").ap()
    d_idn = nc.dram_tensor("idn", [128, 128], BF16, kind="ExternalInput").ap()
    d_onesb = nc.dram_tensor("onesb", [128, 128], BF16,
                             kind="ExternalInput").ap()
    d_onesr = nc.dram_tensor("onesr", [128, 128], F32R,
                             kind="ExternalInput").ap()
    d_out = nc.dram_tensor("out", [T, C], BF16, kind="ExternalOutput").ap()

    with tile.TileContext(nc) as tc:
        with tc.tile_pool(name="glob", bufs=1) as glob:
            # roped q0..q3 / k in bf16, one tile per (comp, t-block)
            QQ = [[glob.tile([128, 512], BF16, name=f"qq{j}_{tb}",
                             tag=f"qq{j}_{tb}")
                   for tb in range(NT)] for j in range(5)]
            # V in [t, hs] layout (bf16), col u = t-chunk
            V = [glob.tile([128, 512], BF16, name=f"v{tb}", tag=f"v{tb}")
                 for tb in range(NT)]
            # proj inputs: fp8 hi/lo, head-paired, one tile per (pair, b)
            YA = [[glob.tile([128, 2, 512], FP8, name=f"ya{i}_{b}",
                             tag=f"ya{i}_{b}") for b in range(NT)]
                  for i in range(2)]
            YB = [[glob.tile([128, 2, 512], FP8, name=f"yb{i}_{b}",
                             tag=f"yb{i}_{b}") for b in range(NT)]
                  for i in range(2)]
            COS = glob.tile([128, T], F32R)
            SIN = glob.tile([128, T], F32R)
            MASK = glob.tile([128, 128], BF16)
            PERM = glob.tile([128, 128], F32R)
            IDN = glob.tile([128, 128], BF16)
            ONESB = glob.tile([128, 128], BF16)
            ONESR = glob.tile([128, 128], F32R)

            def load_consts():
                nc.sync.dma_start(PERM[:], d_perm[:])
                nc.sync.dma_start(IDN[:], d_idn[:])

            def load_consts2():
                nc.sync.dma_start(COS[:], d_cos[:])
                nc.sync.dma_start(SIN[:], d_sin[:])
                nc.sync.dma_start(MASK[:], d_mask[:])
                nc.sync.dma_start(ONESB[:], d_onesb[:])
                nc.sync.dma_start(ONESR[:], d_onesr[:])

            # ---------------- Phase A: qkv projection + rope -------------
            with tc.tile_pool(name="wa", bufs=1) as wap, \
                 tc.tile_pool(name="xp", bufs=2) as xp, \
                 tc.tile_pool(name="tmpa", bufs=2) as tmpa, \
                 tc.tile_pool(name="psA", bufs=1, space="PSUM") as psA, \
                 tc.tile_pool(name="psR", bufs=2, space="PSUM") as psR:
                WH = [wap.tile([128, 8, RG], FP8, name=f"whq{q}",
                               tag=f"wh{q}") for q in range(NQ)]
                WL = [wap.tile([128, 8, RG], FP8, name=f"wlq{q}",
                               tag=f"wl{q}") for q in range(NQ)]

                for tb in range(NT):
                    ts = slice(tb * 512, (tb + 1) * 512)
                    qkv_ps = [psA.tile([128, 512], F32, tag=f"qkv{j}",
                                       name=f"qkv{j}")
                              for j in range(NCOMP)]
                    XH = [xp.tile([128, 8, 512], FP8, name=f"xhq{q}",
                                  tag=f"xh{q}") for q in range(NQ)]
                    XL = [xp.tile([128, 8, 512], FP8, name=f"xlq{q}",
                                  tag=f"xl{q}") for q in range(NQ)]
                    for q in range(NQ):
                        cs = slice(q * 8, (q + 1) * 8)
                        if tb == 0 and q == 0:
                            # first quarter in halves: earliest possible start
                            for hh in range(2):
                                c4 = slice(4 * hh, 4 * hh + 4)
                                nc.sync.dma_start(WH[0][:, c4, :],
                                                  d_wh[:, c4, :])
                                nc.sync.dma_start(XH[0][:, c4, :],
                                                  d_xh[0, :, c4, :])
                            nc.sync.dma_start(WL[0][:], d_wl[:, cs, :])
                            nc.sync.dma_start(XL[0][:], d_xl[0, :, cs, :])
                        else:
                            if tb == 0:
                                nc.sync.dma_start(WH[q][:], d_wh[:, cs, :])
                            nc.sync.dma_start(XH[q][:], d_xh[tb, :, cs, :])
                            if tb == 0:
                                nc.sync.dma_start(WL[q][:], d_wl[:, cs, :])
                            nc.sync.dma_start(XL[q][:], d_xl[tb, :, cs, :])
                        if tb == 0 and q == 0:
                            # consts go after the first compute quarter so
                            # the PE warmup isn't stuck behind them
                            load_consts()
                        if tb == 0 and q == NQ - 1:
                            load_consts2()
                    # quarter-outer, term-grouped: term a's inputs land
                    # first, so its matmuls run while b/c inputs stream in
                    for q in range(NQ):
                        for t_i, (wt, xt) in enumerate(
                                ((WH, XH), (WH, XL), (WL, XH))):
                            for u in range(4):
                                ps = slice(2 * u, 2 * u + 2)
                                for j in range(NCOMP):
                                    js = slice(j * HS, (j + 1) * HS)
                                    nc.tensor.matmul(
                                        qkv_ps[j][:],
                                        wt[q][:, ps, js],
                                        xt[q][:, ps, :],
                                        start=(q == 0 and t_i == 0 and
                                               u == 0),
                                        stop=(q == NQ - 1 and t_i == 2 and
                                              u == 3),
                                        perf_mode=DR)

                    for j in range(5):  # q0..q3, k get rope
                        raw = tmpa.tile([128, 512], F32R, tag="raw")
                        nc.scalar.copy(raw[:], qkv_ps[j][:])
                        rot = psR.tile([128, 512], F32, tag="rot")
                        nc.tensor.matmul(rot[:], PERM[:], raw[:],
                                         start=True, stop=True)
                        t1 = tmpa.tile([128, 512], F32R, tag="t1")
                        nc.vector.tensor_tensor(t1[:], raw[:], COS[:, ts],
                                                mybir.AluOpType.mult)
                        t2 = tmpa.tile([128, 512], F32R, tag="t2")
                        nc.vector.tensor_tensor(t2[:], rot[:], SIN[:, ts],
                                                mybir.AluOpType.mult)
                        nc.vector.tensor_tensor(QQ[j][tb][:], t1[:], t2[:],
                                                mybir.AluOpType.add)

                    # v: scale back by 1/64, transpose [hs, t] -> [t, hs]
                    vraw = tmpa.tile([128, 512], BF16, tag="vraw")
                    nc.scalar.mul(vraw[:], qkv_ps[5][:], 1.0 / WSCALE)
                    for u in range(4):
                        vt = psR.tile([128, 128], BF16, tag="rot")
                        nc.tensor.transpose(vt[:],
                                            vraw[:, u * 128:(u + 1) * 128],
                                            IDN[:])
                        nc.vector.tensor_copy(
                            V[tb][:, u * 128:(u + 1) * 128], vt[:])

            # ---------------- Phase B: causal attention ------------------
            with tc.tile_pool(name="wp", bufs=1) as wpp, \
                 tc.tile_pool(name="pwp", bufs=4) as pwp, \
                 tc.tile_pool(name="prp", bufs=3) as prp, \
                 tc.tile_pool(name="bcp", bufs=2) as bcp, \
                 tc.tile_pool(name="yfp", bufs=2) as yfp, \
                 tc.tile_pool(name="orw", bufs=3) as orw, \
                 tc.tile_pool(name="rcp", bufs=2) as rcp:
                WPH = [wpp.tile([128, 2, C], FP8, name=f"wph{i}",
                                tag=f"wph{i}") for i in range(2)]
                WPL = [wpp.tile([128, 2, C], FP8, name=f"wpl{i}",
                                tag=f"wpl{i}") for i in range(2)]
                for h in range(QPK):
                    nc.sync.dma_start(WPH[h // 2][:, h % 2, :],
                                      d_wph[h * 128:(h + 1) * 128, :])
                    nc.sync.dma_start(WPL[h // 2][:, h % 2, :],
                                      d_wpl[h * 128:(h + 1) * 128, :])

                with tc.tile_pool(name="psS", bufs=2, space="PSUM") as psS, \
                     tc.tile_pool(name="psY", bufs=1, space="PSUM") as psY, \
                     tc.tile_pool(name="psD", bufs=1, space="PSUM") as psD, \
                     tc.tile_pool(name="psO", bufs=2, space="PSUM") as psO:

                    def emit_b_block(b, cwork=None):
                        for h in range(QPK):
                            if cwork is not None:
                                emit_c_tt(cwork[0] * 4 + h)
                            nkt = 4 * (b + 1)
                            y_ps = psY.tile([128, 512], F32, tag="y")
                            d_ps = psD.tile([1, 512], F32, tag="d")
                            first_d = [True]
                            quad = [None]

                            def d_mm(rhs_ap, off, last):
                                nc.tensor.matmul(
                                    d_ps[:, off:], ONESB[:, :1], rhs_ap,
                                    start=first_d[0], stop=last)
                                first_d[0] = False

                            for u in range(nkt // 2):
                                s_w = psS.tile([128, 1024], F32, tag="s")
                                p_w = pwp.tile([128, 1024], BF16, tag="p")
                                offs = []
                                for half in range(2):
                                    kt = 2 * u + half
                                    r = kt - 4 * b
                                    off = 0 if r < 0 else r * 128
                                    offs.append(off)
                                    co = half * 512
                                    nc.tensor.matmul(
                                        s_w[:, co + off:co + 512],
                                        QQ[4][kt // 4][:, (kt % 4) * 128:
                                                       (kt % 4 + 1) * 128],
                                        QQ[h][b][:, off:],
                                        start=True, stop=True)
                                full_pair = offs[0] == 0 and offs[1] == 0
                                if full_pair:
                                    nc.scalar.activation(
                                        p_w[:], s_w[:],
                                        mybir.ActivationFunctionType.Exp,
                                        scale=SCALE)
                                else:
                                    for half in range(2):
                                        co = half * 512 + offs[half]
                                        nc.scalar.activation(
                                            p_w[:, co:half * 512 + 512],
                                            s_w[:, co:half * 512 + 512],
                                            mybir.ActivationFunctionType.Exp,
                                            scale=SCALE)
                                for half in range(2):
                                    kt = 2 * u + half
                                    r = kt - 4 * b
                                    if r >= 0:  # diagonal: mask 128-col strip
                                        co = half * 512 + r * 128
                                        nc.vector.tensor_tensor(
                                            p_w[:, co:co + 128],
                                            p_w[:, co:co + 128],
                                            MASK[:],
                                            mybir.AluOpType.mult)
                                for half in range(2):
                                    kt = 2 * u + half
                                    off = offs[half]
                                    co = half * 512
                                    nc.tensor.matmul(
                                        y_ps[:, off:],
                                        V[kt // 4][:, (kt % 4) * 128:
                                                   (kt % 4 + 1) * 128],
                                        p_w[:, co + off:co + 512],
                                        start=(kt == 0), stop=(kt == nkt - 1))
                                # denominator: quad-sum full tiles on DVE,
                                # one ones-matmul per quad / diagonal half
                                if full_pair:
                                    pr = prp.tile([128, 512], BF16, tag="pr")
                                    nc.vector.tensor_tensor(
                                        pr[:], p_w[:, :512], p_w[:, 512:],
                                        mybir.AluOpType.add)
                                    if u % 2 == 0:
                                        quad[0] = pr
                                    else:
                                        q2 = prp.tile([128, 512], BF16,
                                                      tag="q2")
                                        nc.vector.tensor_tensor(
                                            q2[:], quad[0][:], pr[:],
                                            mybir.AluOpType.add)
                                        quad[0] = None
                                        d_mm(q2[:], 0, False)
                                else:
                                    if quad[0] is not None:
                                        d_mm(quad[0][:], 0, False)
                                        quad[0] = None
                                    for half in range(2):
                                        off = offs[half]
                                        co = half * 512
                                        d_mm(p_w[:, co + off:co + 512], off,
                                             (u == nkt // 2 - 1 and
                                              half == 1))
                            recip = rcp.tile([1, 512], F32R, tag="r")
                            with nc.allow_low_precision(
                                    reason="f32r is f32-width"):
                                nc.vector.reciprocal(recip[:], d_ps[:])
                            bc_ps = psD.tile([128, 512], F32, tag="d")
                            nc.tensor.matmul(bc_ps[:], ONESR[:1, :], recip[:],
                                             start=True, stop=True)
                            bc_sb = bcp.tile([128, 512], BF16, tag="bc")
                            nc.vector.tensor_copy(bc_sb[:], bc_ps[:])
                            yf = yfp.tile([128, 512], F32R, tag="yf")
                            nc.vector.tensor_tensor(yf[:], y_ps[:], bc_sb[:],
                                                    mybir.AluOpType.mult)
                            ya = YA[h // 2][b]
                            yb = YB[h // 2][b]
                            nc.scalar.copy(ya[:, h % 2, :], yf[:])
                            nc.vector.tensor_tensor(
                                yb[:, h % 2, :], yf[:], ya[:, h % 2, :],
                                mybir.AluOpType.subtract)

                    # ------------- Phase C: output projection ------------
                    def emit_c_tt(tt):
                            b = tt // 4
                            tsl = slice((tt % 4) * 128, (tt % 4) * 128 + 128)
                            o_row = orw.tile([128, C], BF16, tag="orow")
                            for cb in range(C // 512):
                                cs = slice(cb * 512, (cb + 1) * 512)
                                o_ps = psO.tile([128, 512], F32, tag="o")
                                terms = [(YA[0][b], WPH[0]),
                                         (YA[1][b], WPH[1]),
                                         (YB[0][b], WPH[0]),
                                         (YB[1][b], WPH[1]),
                                         (YA[0][b], WPL[0]),
                                         (YA[1][b], WPL[1])]
                                for t_i, (yt, wt) in enumerate(terms):
                                    nc.tensor.matmul(
                                        o_ps[:], yt[:, :, tsl], wt[:, :, cs],
                                        start=(t_i == 0), stop=(t_i == 5),
                                        perf_mode=DR)
                                # split psum->sbuf copies across Act and DVE
                                if cb % 2 == 0:
                                    nc.scalar.mul(o_row[:, cs], o_ps[:],
                                                  1.0 / WSCALE)
                                else:
                                    nc.vector.tensor_scalar_mul(
                                        o_row[:, cs], o_ps[:], 1.0 / WSCALE)
                            for dh in range(2):
                                hs_ = slice(dh * 2048, (dh + 1) * 2048)
                                nc.sync.dma_start(
                                    d_out[tt * 128:(tt + 1) * 128, hs_],
                                    o_row[:, hs_])

                    # interleave: C(b-1) emitted after B(b) so its matmuls
                    # can fill PE bubbles while B waits on exp
                    emit_b_block(0)
                    emit_b_block(1)
                    for tt in range(0, 4):
                        emit_c_tt(tt)
                    emit_b_block(2)
                    for tt in range(4, 8):
                        emit_c_tt(tt)
                    emit_b_block(3)
                    for tt in range(8, 12):
                        emit_c_tt(tt)

                with tc.tile_pool(name="psO2", bufs=6, space="PSUM") as psO2:
                    for tt in range(12, 16):
                        b = tt // 4
                        tsl = slice((tt % 4) * 128, (tt % 4) * 128 + 128)
                        o_row = orw.tile([128, C], BF16, tag="orow")
                        for cb in range(C // 512):
                            cs = slice(cb * 512, (cb + 1) * 512)
                            o_ps = psO2.tile([128, 512], F32, tag="o")
                            terms = [(YA[0][b], WPH[0]),
                                     (YA[1][b], WPH[1]),
                                     (YB[0][b], WPH[0]),
                                     (YB[1][b], WPH[1]),
                                     (YA[0][b], WPL[0]),
                                     (YA[1][b], WPL[1])]
                            for t_i, (yt, wt) in enumerate(terms):
                                nc.tensor.matmul(
                                    o_ps[:], yt[:, :, tsl], wt[:, :, cs],
                                    start=(t_i == 0), stop=(t_i == 5),
                                    perf_mode=DR)
                            if cb % 2 == 0:
                                nc.scalar.mul(o_row[:, cs], o_ps[:],
                                              1.0 / WSCALE)
                            else:
                                nc.vector.tensor_scalar_mul(
                                    o_row[:, cs], o_ps[:], 1.0 / WSCALE)
                        for dh in range(4):
                            hs_ = slice(dh * 1024, (dh + 1) * 1024)
                            nc.sync.dma_start(
                                d_out[tt * 128:(tt + 1) * 128, hs_],
                                o_row[:, hs_])
    nc.compile()
    return nc


def _host_inputs(x, cos, sin, w_attn, w_proj):
    """Build per-core input maps (host-side shard/transpose/fp8 split)."""
    f = np.float32
    f8 = ml_dtypes.float8_e4m3fn

    def split8(a):
        hi = a.astype(f8)
        lo = (a - hi.astype(f)).astype(f8)
        return hi, lo

    xt = x.reshape(T, C).T.astype(f)                         # [C, T]
    xh, xl = split8(xt)

    def xlayout(a):
        # [C, T] -> [tb, p, n, c] with C = n*128+p, T = tb*512+c
        return np.ascontiguousarray(
            a.reshape(NC, 128, NT, 512).transpose(2, 1, 0, 3))

    xh, xl = xlayout(xh), xlayout(xl)
    cost = np.ascontiguousarray(cos.T.astype(f) / WSCALE)    # [HS, T]
    sgn = np.ones((HS, 1), f)
    sgn[:HS // 2] = -1.0
    sint = np.ascontiguousarray((sin.T * sgn).astype(f) / WSCALE)
    # rot(x)=P@x in [d,t] layout; matmul computes lhsT.T @ rhs -> lhsT = P.T
    P = np.zeros((HS, HS), f)
    for i in range(HS // 2):
        P[i, i + HS // 2] = 1.0
        P[i + HS // 2, i] = 1.0
    perm = np.ascontiguousarray(P.T)
    idn = np.eye(128, dtype=f).astype(ml_dtypes.bfloat16)
    onesb = np.ones((128, 128), ml_dtypes.bfloat16)
    onesr = np.ones((128, 128), f)
    # causal mask strip: within a diagonal 128x128 tile, q offset i vs key
    # partition p: keep iff i >= p (same strip for every diagonal offset r)
    pidx = np.arange(128)
    cols = np.arange(128)
    mask = (cols[None, :] >= pidx[:, None]).astype(f).astype(
        ml_dtypes.bfloat16)

    def wlayout(a):
        # [C, RG] -> [p, n, col]
        return np.ascontiguousarray(
            a.reshape(NC, 128, RG).transpose(1, 0, 2))

    maps = []
    for g in range(N_CORES):
        wat = (w_attn[g * RG:(g + 1) * RG, :].T.astype(f) * WSCALE)  # [C, RG]
        wh, wl = split8(wat)
        wh, wl = wlayout(wh), wlayout(wl)
        wpt = (w_proj[:, g * OG:(g + 1) * OG].T.astype(f) * WSCALE)  # [OG, C]
        wph, wpl = split8(wpt)
        maps.append({
            "xh": xh, "xl": xl, "wh": wh, "wl": wl,
            "wph": np.ascontiguousarray(wph),
            "wpl": np.ascontiguousarray(wpl),
            "cost": cost, "sint": sint, "mask": mask, "perm": perm,
            "idn": idn, "onesb": onesb, "onesr": onesr,
        })
    return maps


_PROGRAM = None


def kernel(x, cos, sin, w_attn, w_proj):
    global _PROGRAM
    if _PROGRAM is None:
        _PROGRAM = _build_program()
    maps = _host_inputs(np.asarray(x), np.asarray(cos), np.asarray(sin),
                        np.asarray(w_attn), np.asarray(w_proj))
    res = bass_utils.run_bass_kernel_spmd(_PROGRAM, maps, list(range(N_CORES)))
    out = np.zeros((T, C), np.float32)
    for g in range(N_CORES):
        out += np.asarray(res.results[g]["out"]).astype(np.float32)
    return out.reshape(1, T, C)

